# revision 43
# baseline (speedup 1.0000x reference)
"""EquivariantBlock Trainium kernel v2: bf16 + block-aligned chunking.

Layout / sharding:
  - 8 cores, data-parallel by target-node range (2500 nodes each, NRP=2560
    padded). Node tables replicated but ROLLED per core so the core's own
    range sits at rows [0, 2560).
  - Device preprocess (per core, replicated over all NP nodes): per-graph
    centering + E3Norm of X -> XP (bf16), LayerNorm of H -> HLN (bf16,
    gamma/beta folded into MLP weights on host), stored as one record table
    XH = [XP | HLN] ([NP, 448] bf16) in DRAM.
  - Edges sorted by target, grouped into chunks of CH=512 edges where each
    chunk's targets lie inside ONE aligned 128-node block. Per-block chunk
    counts are maxed across cores so all 8 cores share one program.
  - Per chunk: gather src records (bf16, 896B each); target side comes from
    a sequential per-block load + one-hot expansion matmuls (no tgt gather).
    rel is accumulated on the PE (identity matmul + negated-block expand).
    MLP runs feature-major at bf16 (1 cyc/row). Per-128-edge-group one-hot
    segment-sum matmuls accumulate the update in PSUM across all chunks of
    the block; OUT = upsum + XP_block written directly (no UPD table).
  - Act engine table discipline: preproc uses only sqrt-table funcs; the
    edge loop runs a sqrt phase (S) then a silu phase (M) per block, so the
    compiler inserts only 2 act-table loads per block.
"""
import numpy as np
import ml_dtypes

BF16 = ml_dtypes.bfloat16

F = 64
K = 128
XW = 3 * K          # 384
REC = XW + F        # 448
RECG = 512          # gather record (padded: 512*2B is a multiple of 256)
CH = 512            # edges per chunk
G = CH // 128       # groups per chunk
EPS_E3 = 1e-5
EPS_LN = 1e-5
CLAMP = 10.0


def bf(a):
    return np.ascontiguousarray(np.asarray(a, np.float32).astype(BF16))


def bfr(a):
    """Round to bf16, return fp32 (for emulation)."""
    return np.asarray(a, np.float32).astype(BF16).astype(np.float32)


class Cfg:
    def __init__(self, n_nodes, n_graphs, cores):
        self.N = n_nodes
        self.NG = n_graphs
        self.CORES = cores
        self.NR = n_nodes // cores        # nodes per core
        self.NRP = -(-self.NR // 128) * 128
        self.NB = self.NRP // 128         # target blocks per core
        self.NCHK = -(-n_nodes // 128)    # node chunks for preprocess
        self.NP = self.NCHK * 128


CFG_FULL = Cfg(20000, 64, 8)


# ---------------------------------------------------------------- host prep

def build_shards(cfg, src, tgt, edge_attr, te):
    """Partition edges by target block; schedule shared across cores."""
    N, NR, NB, CORES = cfg.N, cfg.NR, cfg.NB, cfg.CORES
    percore = []
    for c in range(CORES):
        em = np.where(np.minimum(tgt // NR, CORES - 1) == c)[0]
        tl0 = (tgt[em] - c * NR).astype(np.int64)
        nr_here = N - c * NR if c == CORES - 1 else NR
        deg = np.bincount(tl0, minlength=NR)
        # greedy balance: assign nodes (desc degree) to blocks, cap 128
        # nodes and minimal edge total per block -> all blocks ~N_edges/NB
        order_nodes = np.argsort(-deg, kind="stable")
        blk_of = np.zeros(NR, np.int64)
        slot_of = np.zeros(NR, np.int64)
        btot = np.zeros(NB, np.int64)
        bcnt = np.zeros(NB, np.int64)
        for v in order_nodes:
            cand = np.where(bcnt < 128)[0]
            bsel = cand[np.argmin(btot[cand])]
            blk_of[v] = bsel
            slot_of[v] = bcnt[bsel]
            btot[bsel] += deg[v]
            bcnt[bsel] += 1
        perm = blk_of * 128 + slot_of          # old local id -> new local id
        iperm = np.zeros(cfg.NRP, np.int64)
        iperm[perm] = np.arange(NR)            # new local id -> old local id
        tl = perm[tl0]
        order = np.argsort(tl, kind="stable")
        eidx = em[order]
        tl = tl[order]
        sg = src[eidx]
        blk = tl // 128
        cnt = np.bincount(blk, minlength=NB)
        percore.append(dict(eidx=eidx, tl=tl, sg=sg, cnt=cnt,
                            perm=perm, iperm=iperm))

    # shared schedule: chunks per block = max over cores
    nchb = [max(1, int(-(-max(pc["cnt"][b] for pc in percore) // CH)))
            for b in range(NB)]
    sched = tuple(nchb)
    NCH = sum(nchb)
    LE = NCH * CH

    shards = []
    for c in range(CORES):
        pc = percore[c]
        eidx, tl, sg, cnt = pc["eidx"], pc["tl"], pc["sg"], pc["cnt"]
        perm = pc["perm"]
        starts = np.concatenate([[0], np.cumsum(cnt)])
        sidx = np.zeros(LE, np.int64)
        loc = np.zeros(LE, np.int64)
        epos = np.full(LE, -1, np.int64)
        pos = 0
        for b in range(NB):
            e0, e1 = starts[b], starts[b + 1]
            ne = e1 - e0
            cap = nchb[b] * CH
            assert ne <= cap, f"block {b} core {c}: {ne} > {cap}"
            sl = slice(pos, pos + ne)
            sr = (sg[e0:e1] - c * NR) % N      # rolled row of source
            own = sr < NR
            sr = np.where(own, 0, sr + (cfg.NRP - NR))
            sr[own] = perm[((sg[e0:e1] - c * NR) % N)[own]]
            sidx[sl] = sr
            loc[sl] = tl[e0:e1] - b * 128
            epos[sl] = eidx[e0:e1]
            dl = slice(pos + ne, pos + cap)
            sidx[dl] = b * 128
            loc[dl] = 0
            pos += cap
        assert pos == LE

        ef = np.zeros((128, LE), np.float32)
        valid = epos >= 0
        ef[:F, valid] = edge_attr[epos[valid]].T
        ef[F:, valid] = te[epos[valid]].T

        def wrap16(v):
            return np.ascontiguousarray(
                np.tile(v.astype(np.int16).reshape(-1, 16).T, (8, 1)))

        shards.append(dict(
            sidx16=wrap16(sidx), perm=perm, iperm=pc["iperm"],
            locp=np.ascontiguousarray(
                loc.reshape(-1, 128).T.astype(np.float32)),
            locr=bf(loc.reshape(NCH, CH)),              # [NCH, 512]
            ef=bf(ef),
            sidx=sidx, loc=loc, epos=epos,
        ))
    return shards, sched


def make_params(cfg, Wm1, bm1, Wm2, bm2, Wx1, bx1, Wx2, bx2, ln_gamma, ln_beta,
                e3_weight):
    f = np.float32
    Wm1 = np.asarray(Wm1, f)
    g = np.asarray(ln_gamma, f).reshape(F)
    bt = np.asarray(ln_beta, f).reshape(F)
    W_ht = Wm1[0:F] * g[:, None]          # fold LN gamma into H weights
    W_hs = Wm1[F:2 * F] * g[:, None]
    b1 = (np.asarray(bm1, f).reshape(F)
          + bt @ Wm1[0:F] + bt @ Wm1[F:2 * F])   # fold LN beta
    b3 = (np.asarray(bx1, f).reshape(F)
          + np.asarray(bm2, f).reshape(F) @ np.asarray(Wx1, f))  # fold bm2
    return dict(
        whs=bf(W_hs),
        wht=bf(W_ht),
        weate=bf(np.concatenate([Wm1[2 * F:3 * F], Wm1[2 * F + K + F:]], 0)),
        wrd=bf(Wm1[3 * F:3 * F + K]),                       # [128, 64]
        w23=bf(np.asarray(Wm2, f) @ np.asarray(Wx1, f)),
        wx2=bf(Wx2),
        bm1=np.ascontiguousarray(b1.reshape(F, 1), f),
        b3=np.ascontiguousarray(b3.reshape(F, 1), f),
        bx2=np.ascontiguousarray(np.asarray(bx2, f).reshape(K, 1), f),
        e3k=np.ascontiguousarray(np.asarray(e3_weight, f).reshape(1, K), f),
    )


def make_onehots(cfg, bf_ids):
    NGP = cfg.NG
    oh = (bf_ids[:, None] == np.arange(NGP)[None, :]).astype(np.float32)
    ohp = np.ascontiguousarray(
        oh.reshape(cfg.NCHK, 128, NGP).transpose(1, 0, 2).reshape(128, -1))
    oht = np.ascontiguousarray(oh.T)
    return bf(ohp), bf(oht), NGP


def prep_core_inputs(cfg, c, shard, batch, X, H, params):
    N, NR, NP, NRP = cfg.N, cfg.NR, cfg.NP, cfg.NRP
    roll = lambda a: np.roll(a, -c * NR, axis=0)
    Xp = np.zeros((NP, XW), np.float32)
    Xp[:N] = roll(np.asarray(X, np.float32).reshape(N, XW))
    Hp = np.zeros((NP, F), np.float32)
    Hp[:N] = roll(np.asarray(H, np.float32))
    bf_ids = np.full(NP, -1.0, np.float32)
    bf_ids[:N] = roll(np.asarray(batch)).astype(np.float32)
    # permute own range into degree-balanced block order; shift the rest
    # of the rolled table up by NRP-NR rows so nothing is clobbered
    perm = shard["perm"]
    off = NRP - NR
    for arr, fill in ((Xp, 0.0), (Hp, 0.0), (bf_ids, -1.0)):
        ownnew = np.full((NRP,) + arr.shape[1:], fill, arr.dtype)
        ownnew[perm] = arr[:NR].copy()
        rest = arr[NR:NP - off].copy()
        arr[:NRP] = ownnew
        arr[NRP:NP] = rest
    ohp, oht, NGP = make_onehots(cfg, bf_ids)
    cnts = np.zeros(NGP, np.float64)
    gg, n = np.unique(np.asarray(batch), return_counts=True)
    cnts[gg.astype(np.int64)] = n
    invc_h = (1.0 / np.maximum(cnts, 1.0)).astype(np.float32).reshape(NGP, 1)
    m = dict(X=bf(Xp), H=Hp, ohp=ohp, oht=oht, invc=invc_h,
             sidx=shard["sidx16"], locp=shard["locp"], locr=shard["locr"],
             EF=shard["ef"])
    m.update(params)
    return m


# ---------------------------------------------------------------- device program

def build_program(cfg, sched, num_devices):
    import concourse.bacc as bacc
    import concourse.bass as bass
    import concourse.tile as tile
    from concourse import mybir
    from concourse.masks import make_identity
    from concourse.tile import add_dep_helper

    def dep(a, b, why):
        add_dep_helper(a.ins, b.ins, sync=True, reason=why)

    f32 = mybir.dt.float32
    bf16 = mybir.dt.bfloat16
    i16 = mybir.dt.int16
    AF = mybir.ActivationFunctionType
    OP = mybir.AluOpType

    N, NP, NCHK, NRP, NB = cfg.N, cfg.NP, cfg.NCHK, cfg.NRP, cfg.NB
    NGP = cfg.NG
    nchb = list(sched)
    NCH = sum(nchb)
    LE = NCH * CH
    NCB = max(nchb)

    nc = bacc.Bacc("TRN2", target_bir_lowering=False, debug=False,
                   num_devices=num_devices)

    def din(name, shape, dt=bf16):
        return nc.dram_tensor(name, shape, dt, kind="ExternalInput").ap()

    X = din("X", [NP, XW])
    H = din("H", [NP, F], f32)
    OHP = din("ohp", [128, NCHK * NGP])
    OHT = din("oht", [NGP, NP])
    INVC = din("invc", [NGP, 1], f32)
    SIDX = din("sidx", [128, LE // 16], i16)
    LOCP = din("locp", [128, LE // 128], f32)
    LOCR = din("locr", [NCH, CH])
    EF = din("EF", [128, LE])
    WHS = din("whs", [F, F])
    WHT = din("wht", [F, F])
    WEATE = din("weate", [2 * F, F])
    WRD = din("wrd", [K, F])
    W23 = din("w23", [F, F])
    WX2 = din("wx2", [F, K])
    BM1 = din("bm1", [F, 1], f32)
    B3 = din("b3", [F, 1], f32)
    BX2 = din("bx2", [K, 1], f32)
    E3K = din("e3k", [1, K], f32)

    OUT = nc.dram_tensor("OUT", [NRP, XW], f32, kind="ExternalOutput").ap()

    def bcast(dram_ap, parts):
        return bass.AP(tensor=dram_ap.tensor, offset=dram_ap.offset,
                       ap=[[0, parts]] + [list(p) for p in dram_ap.ap[1:]])

    def rep_mid(ap, n):
        """[P, W] -> [P, n, W] with a stride-0 middle dim."""
        aps = [list(p) for p in ap.ap]
        return bass.AP(tensor=ap.tensor, offset=ap.offset,
                       ap=aps[:-1] + [[0, n]] + aps[-1:])

    import contextlib
    with tile.TileContext(nc) as tc, \
         nc.allow_low_precision(reason="bf16 kernel, 2e-2 tolerance"), \
         contextlib.ExitStack() as ctx:
        if True:
            const = ctx.enter_context(tc.tile_pool(name="const", bufs=1))
            dramp = ctx.enter_context(tc.tile_pool(name="dram", bufs=1, space="DRAM"))

            XH = dramp.tile([NP, RECG], bf16)

            # ---- constants
            identb = const.tile([128, 128], bf16)
            make_identity(nc, identb[:])
            iotacol_i = const.tile([128, 1], mybir.dt.int32)
            nc.gpsimd.iota(iotacol_i[:], pattern=[[1, 1]], base=0,
                           channel_multiplier=1)
            iotacol = const.tile([128, 1], f32)
            nc.vector.tensor_copy(iotacol[:], iotacol_i[:])
            iotarow_i = const.tile([128, 128], mybir.dt.int32)
            nc.gpsimd.iota(iotarow_i[:], pattern=[[1, 128]], base=0,
                           channel_multiplier=0)
            iotarow = const.tile([128, 128], bf16)
            nc.vector.tensor_copy(iotarow[:], iotarow_i[:])

            whs = const.tile([F, F], bf16)
            nc.sync.dma_start(out=whs[:], in_=WHS)
            wht = const.tile([F, F], bf16)
            nc.sync.dma_start(out=wht[:], in_=WHT)
            weate = const.tile([2 * F, F], bf16)
            nc.sync.dma_start(out=weate[:], in_=WEATE)
            wrd = const.tile([K, F], bf16)
            nc.sync.dma_start(out=wrd[:], in_=WRD)
            w23 = const.tile([F, F], bf16)
            nc.sync.dma_start(out=w23[:], in_=W23)
            wx2 = const.tile([F, K], bf16)
            nc.sync.dma_start(out=wx2[:], in_=WX2)
            bm1c = const.tile([F, 1], f32)
            nc.sync.dma_start(out=bm1c[:], in_=BM1)
            b3c = const.tile([F, 1], f32)
            nc.sync.dma_start(out=b3c[:], in_=B3)
            bx2c = const.tile([K, 1], f32)
            nc.sync.dma_start(out=bx2c[:], in_=BX2)
            e3b = const.tile([NGP, K], f32)
            nc.sync.dma_start(out=e3b[:], in_=bcast(E3K, NGP))
            eps8 = const.tile([128, 1], f32)
            nc.vector.memset(eps8[:], 1e-8)
            epsln = const.tile([128, 1], f32)
            nc.vector.memset(epsln[:], EPS_LN)

            Mneg = const.tile([NGP, XW], bf16)
            sfb = const.tile([NGP, K], bf16)
            invc = const.tile([NGP, 1], f32)
            zpad8 = const.tile([128, 8, RECG - REC], bf16)
            nc.vector.memset(zpad8[:], 0.0)

            # ---- preprocessing (sqrt act-table only)
            with tc.tile_pool(name="xall", bufs=1) as xap, \
                 tc.tile_pool(name="preoh", bufs=1) as poh, \
                 tc.tile_pool(name="pre", bufs=3) as pre, \
                 tc.tile_pool(name="prep", bufs=2, space="PSUM") as pps, \
                 tc.tile_pool(name="preacc", bufs=1, space="PSUM") as pacc:

                ohp_sb = poh.tile([128, NCHK * NGP], bf16)
                nc.sync.dma_start(out=ohp_sb[:], in_=OHP)
                nc.sync.dma_start(out=invc[:], in_=INVC)

                groups = [(cb, min(4, NCHK - cb)) for cb in range(0, NCHK, 4)]

                xall = xap.tile([128, NCHK, XW], bf16)
                for cb, nb in groups:
                    nc.sync.dma_start(
                        out=xall[:, cb:cb + nb, :],
                        in_=X[cb * 128:(cb + nb) * 128, :].rearrange(
                            "(c p) w -> p c w", p=128))

                # P1: per-graph sums of X -> M_mean (negated bf16)
                ps_m = pacc.tile([NGP, XW], f32, space="PSUM")
                for ci in range(NCHK):
                    nc.tensor.matmul(ps_m[:],
                                     ohp_sb[:, ci * NGP:(ci + 1) * NGP],
                                     xall[:, ci, :], start=(ci == 0),
                                     stop=(ci == NCHK - 1))
                mmf = pre.tile([NGP, XW], f32, tag="mmf")
                nc.vector.tensor_scalar_mul(mmf[:], ps_m[:], invc[:, 0:1])
                nc.scalar.activation(Mneg[:], mmf[:], AF.Copy, scale=-1.0)

                # P2: per-graph mean vector-norm (ops batched over 4 chunks)
                ps_n = pacc.tile([NGP, K], f32, space="PSUM")
                for cb, nb in groups:
                    oht4 = pre.tile([NGP, 4, 128], bf16, tag="oht4")
                    nc.sync.dma_start(
                        out=oht4[:, :nb, :],
                        in_=OHT[:, cb * 128:(cb + nb) * 128].rearrange(
                            "g (c p) -> g c p", p=128))
                    xc4 = pps.tile([128, 4, 512], f32, space="PSUM", tag="xc",
                                   bufs=1)
                    for j in range(nb):
                        ci = cb + j
                        nc.tensor.matmul(xc4[:, j, 0:XW], oht4[:, j, :],
                                         Mneg[:], start=True, stop=False)
                        nc.tensor.matmul(xc4[:, j, 0:XW], identb[:],
                                         xall[:, ci, :],
                                         start=False, stop=True)
                    sq4 = pre.tile([128, 4, XW], bf16, tag="sq")
                    nc.scalar.activation(sq4[:, :nb, :], xc4[:, :nb, 0:XW],
                                         AF.Square)
                    nsq4 = pre.tile([128, 4, K], bf16, tag="nsq")
                    nc.vector.tensor_add(nsq4[:, :nb, :], sq4[:, :nb, 0:K],
                                         sq4[:, :nb, K:2 * K])
                    nc.vector.tensor_add(nsq4[:, :nb, :], nsq4[:, :nb, :],
                                         sq4[:, :nb, 2 * K:])
                    nrm4 = pre.tile([128, 4, K], bf16, tag="nrm")
                    nc.scalar.activation(nrm4[:, :nb, :], nsq4[:, :nb, :],
                                         AF.Sqrt)
                    for j in range(nb):
                        ci = cb + j
                        nc.tensor.matmul(ps_n[:],
                                         ohp_sb[:, ci * NGP:(ci + 1) * NGP],
                                         nrm4[:, j, :], start=(ci == 0),
                                         stop=(ci == NCHK - 1))
                mn = pre.tile([NGP, K], f32, tag="mn")
                nc.vector.tensor_scalar(mn[:], ps_n[:], invc[:, 0:1], EPS_E3,
                                        op0=OP.mult, op1=OP.add)
                rmn = pre.tile([NGP, K], f32, tag="rmn")
                nc.vector.reciprocal(rmn[:], mn[:])
                sff = pre.tile([NGP, K], f32, tag="sff")
                nc.vector.tensor_mul(sff[:], rmn[:], e3b[:])
                nc.scalar.activation(sfb[:], sff[:], AF.Copy)

                # P3: XP = (X - M[g]) * sfac[g]  -> XH[:, :XW]
                for cb, nb in groups:
                    oht4 = pre.tile([NGP, 4, 128], bf16, tag="oht4")
                    nc.sync.dma_start(
                        out=oht4[:, :nb, :],
                        in_=OHT[:, cb * 128:(cb + nb) * 128].rearrange(
                            "g (c p) -> g c p", p=128))
                    xp4 = pre.tile([128, 4, XW], bf16, tag="xp4")
                    xc4 = pps.tile([128, 4, 512], f32, space="PSUM", tag="xc",
                                   bufs=1)
                    sexp4 = pps.tile([128, 4, K], f32, space="PSUM",
                                     tag="sexp", bufs=1)
                    for j in range(nb):
                        ci = cb + j
                        nc.tensor.matmul(xc4[:, j, 0:XW], oht4[:, j, :],
                                         Mneg[:], start=True, stop=False)
                        nc.tensor.matmul(xc4[:, j, 0:XW], identb[:],
                                         xall[:, ci, :],
                                         start=False, stop=True)
                        nc.tensor.matmul(sexp4[:, j, :], oht4[:, j, :],
                                         sfb[:], start=True, stop=True)
                    sxb4 = pre.tile([128, 4, K], bf16, tag="sxb")
                    nc.vector.tensor_copy(sxb4[:, :nb, :], sexp4[:, :nb, :])
                    for j in range(nb):
                        nc.vector.scalar_tensor_tensor(
                            xp4[:, j, :].rearrange("p (d k) -> p d k", d=3),
                            xc4[:, j, 0:XW].rearrange("p (d k) -> p d k", d=3),
                            0.0, rep_mid(sxb4[:, j, :], 3),
                            op0=OP.bypass, op1=OP.mult)
                    nc.sync.dma_start(
                        out=XH[cb * 128:(cb + nb) * 128, 0:XW].rearrange(
                            "(c p) w -> p c w", p=128),
                        in_=xp4[:, :nb, :])

                # P4: HLN (no gamma/beta: folded into weights) -> XH[:, XW:]
                hgroups = [(cb, min(8, NCHK - cb)) for cb in range(0, NCHK, 8)]
                for cb, nb in hgroups:
                    h8 = pre.tile([128, 8, F], f32, tag="h8")
                    nc.sync.dma_start(
                        out=h8[:, :nb, :],
                        in_=H[cb * 128:(cb + nb) * 128, :].rearrange(
                            "(c p) w -> p c w", p=128))
                    hg8 = pre.tile([128, 8, F], bf16, tag="hg8")
                    for j in range(nb):
                        ht = h8[:, j, :]
                        st = pre.tile([128, 6], f32, tag="st")
                        nc.vector.bn_stats(out=st[:], in_=ht)
                        mv = pre.tile([128, 2], f32, tag="mv")
                        nc.vector.bn_aggr(out=mv[:], in_=st[:])
                        sd = pre.tile([128, 1], f32, tag="sd")
                        nc.scalar.activation(sd[:], mv[:, 1:2], AF.Sqrt,
                                             bias=epsln[:])
                        rs = pre.tile([128, 1], f32, tag="rs")
                        nc.vector.reciprocal(rs[:], sd[:])
                        nc.vector.tensor_scalar(hg8[:, j, :], ht, mv[:, 0:1],
                                                rs[:, 0:1],
                                                op0=OP.subtract, op1=OP.mult)
                    nc.sync.dma_start(
                        out=XH[cb * 128:(cb + nb) * 128, XW:REC].rearrange(
                            "(c p) w -> p c w", p=128),
                        in_=hg8[:, :nb, :])
                    nc.sync.dma_start(
                        out=XH[cb * 128:(cb + nb) * 128, REC:RECG].rearrange(
                            "(c p) w -> p c w", p=128),
                        in_=zpad8[:, :nb, :])

            # ---- edge loop
            with tc.tile_pool(name="edi", bufs=1) as edi, \
                 tc.tile_pool(name="blk", bufs=2) as blkp, \
                 tc.tile_pool(name="edg", bufs=3) as edg, \
                 tc.tile_pool(name="eds", bufs=2) as eds, \
                 tc.tile_pool(name="keep", bufs=NCB + 2) as keep, \
                 tc.tile_pool(name="psx", bufs=1, space="PSUM") as psx, \
                 tc.tile_pool(name="psht", bufs=1, space="PSUM") as psht, \
                 tc.tile_pool(name="psz", bufs=2, space="PSUM") as psz, \
                 tc.tile_pool(name="psu", bufs=1, space="PSUM") as psu:

                sidx_sb = edi.tile([128, LE // 16], i16)
                nc.sync.dma_start(out=sidx_sb[:], in_=SIDX)
                locp_sb = edi.tile([128, LE // 128], f32)
                nc.sync.dma_start(out=locp_sb[:], in_=LOCP)

                # preprocess XH writes land before gathers (invisible APs)
                tc.strict_bb_all_engine_barrier()

                IC = CH // 16
                gidx_reg = nc.gpsimd.alloc_register("gidx")
                nc.gpsimd.reg_mov(gidx_reg, CH)
                gath_consumers = {}
                pending = {}
                GB = 3  # xhs ring depth

                def issue_gather(cch):
                    war = gath_consumers.pop(cch - GB, None)
                    xhs = edg.tile([128, G, RECG], bf16, tag="xhs",
                                   name=f"xhs{cch}")
                    g1 = nc.gpsimd.dma_gather(
                        out_ap=xhs[:], in_ap=XH[:],
                        idxs_ap=sidx_sb[:, cch * IC:(cch + 1) * IC],
                        num_idxs=CH, num_idxs_reg=gidx_reg, elem_size=RECG,
                        single_packet=False)
                    if war:
                        for ci in war:
                            dep(g1, ci, "war-xhs")
                    pending[cch] = (xhs, g1)

                chunk_base = 0
                prev_last_a3 = [None]
                last_sx = [None]
                for b in range(NB):
                    nchunks = nchb[b]
                    c0 = chunk_base

                    xhtb = blkp.tile([128, RECG], bf16, tag="xhtb")
                    nc.sync.dma_start(out=xhtb[:],
                                      in_=XH[b * 128:(b + 1) * 128, :])
                    # HW = Hblk @ W_ht  (per block; ht-term enters z1 via sel2)
                    hbtp = psht.tile([F, CH], bf16, space="PSUM", tag="hsp")
                    nc.tensor.transpose(hbtp[:, 0:128], xhtb[:, XW:REC],
                                        identb[:])
                    hbt = blkp.tile([F, 128], bf16, tag="hbt")
                    nc.scalar.activation(hbt[:], hbtp[:, 0:128], AF.Copy)
                    hwp = psz.tile([128, CH], f32, space="PSUM", tag="z")
                    nc.tensor.matmul(hwp[:, 0:F], hbt[:], wht[:],
                                     start=True, stop=True)
                    hwb = blkp.tile([128, F], bf16, tag="hwb")
                    nc.scalar.activation(hwb[:], hwp[:, 0:F], AF.Copy)

                    upsum = psu.tile([128, XW], f32, space="PSUM", tag="u",
                                     bufs=2)

                    # ---------------- phase S (sqrt table)
                    sdata = []
                    for kk in range(nchunks):
                        cch = c0 + kk
                        if cch == 0:
                            issue_gather(0)
                            issue_gather(1)
                        if cch + 2 < NCH:
                            issue_gather(cch + 2)
                        xhs, g1 = pending.pop(cch)
                        consumers = []

                        locrep = edg.tile([128, CH], bf16, tag="locrep")
                        nc.sync.dma_start(out=locrep[:],
                                          in_=bcast(LOCR[cch:cch + 1, :], 128))
                        sel = keep.tile([128, G, 128], bf16, tag="sel")
                        sel2 = keep.tile([128, G, 128], bf16, tag="sel2")
                        rel = keep.tile([128, G, XW], bf16, tag="rel")
                        rdh = keep.tile([128, G, K], bf16, tag="rdh")
                        hsT = keep.tile([F, CH], bf16, tag="hsT")
                        rdT = keep.tile([128, CH], bf16, tag="rdT")
                        fd = keep.tile([128, G, K], bf16, tag="fd")

                        # sel[e, l] = (l == loc_e)   (Pool, per group)
                        for g in range(G):
                            nc.gpsimd.tensor_scalar(
                                sel[:, g, :], iotarow[:],
                                locp_sb[:, cch * G + g:cch * G + g + 1],
                                None, op0=OP.is_equal)
                        # sel2[l, (g,e)] = (l == loc_e)   (DVE, one op)
                        nc.vector.tensor_scalar(
                            sel2[:], locrep[:].rearrange("p (g e) -> p g e",
                                                         g=G),
                            iotacol[:, 0:1], None, op0=OP.is_equal)

                        # xpt[(g,e), :] = XP[loc_e]  (PE expand; bank-
                        # aligned 512-padded groups, two groups per substep)
                        for ss in range(G // 2):
                            xpt = psx.tile([128, 2, 512], f32, space="PSUM",
                                           tag="xpt", bufs=1)
                            for g2 in range(2):
                                g = ss * 2 + g2
                                nc.tensor.matmul(xpt[:, g2, 0:XW],
                                                 sel2[:, g, :],
                                                 xhtb[:, 0:XW],
                                                 start=True, stop=True)
                            i_rel = nc.vector.tensor_sub(
                                rel[:, ss * 2:ss * 2 + 2, :],
                                xhs[:, ss * 2:ss * 2 + 2, 0:XW],
                                xpt[:, :, 0:XW])
                            dep(i_rel, g1, "raw-xhs")
                            consumers.append(i_rel)
                        # rd = sum_c rel^2
                        sq = eds.tile([128, G, XW], bf16, tag="sq")
                        nc.scalar.activation(sq[:], rel[:], AF.Square)
                        nc.vector.tensor_add(rdh[:], sq[:, :, 0:K],
                                             sq[:, :, K:2 * K])
                        nc.vector.tensor_add(rdh[:], rdh[:], sq[:, :, 2 * K:])
                        # hsT
                        hsp = psht.tile([F, CH], bf16, space="PSUM", tag="hsp")
                        for g in range(G):
                            i_t = nc.tensor.transpose(
                                hsp[:, g * 128:(g + 1) * 128],
                                xhs[:, g, XW:REC], identb[:])
                            dep(i_t, g1, "raw-xhs-h")
                            consumers.append(i_t)
                        nc.scalar.activation(hsT[:], hsp[:], AF.Copy)
                        # rdT
                        rdp = psht.tile([128, CH], bf16, space="PSUM",
                                        tag="tp")
                        for g in range(G):
                            nc.tensor.transpose(rdp[:, g * 128:(g + 1) * 128],
                                                rdh[:, g, :], identb[:])
                        nc.scalar.activation(rdT[:], rdp[:], AF.Copy)
                        # fach = 1 / (1 + sqrt(rd + 1e-8))
                        sxh = eds.tile([128, G, K], bf16, tag="sxh")
                        i_sx = nc.scalar.activation(sxh[:], rdh[:], AF.Sqrt,
                                                    bias=eps8[:])
                        if prev_last_a3[0] is not None:
                            dep(i_sx, prev_last_a3[0], "act-table-phase")
                        last_sx[0] = i_sx
                        fdt = eds.tile([128, G, K], bf16, tag="fdt")
                        nc.vector.tensor_scalar_add(fdt[:], sxh[:], 1.0)
                        nc.vector.reciprocal(fd[:], fdt[:])
                        gath_consumers[cch] = consumers
                        sdata.append((rel, hsT, rdT, fd, sel, sel2))

                    # ---------------- phase M (silu table)
                    for kk in range(nchunks):
                        cch = c0 + kk
                        rel, hsT, rdT, fd, sel, sel2 = sdata[kk]
                        ef = edg.tile([128, CH], bf16, tag="ef")
                        nc.sync.dma_start(out=ef[:],
                                          in_=EF[:, cch * CH:(cch + 1) * CH])

                        z1 = psz.tile([128, CH], f32, space="PSUM", tag="z")
                        nc.tensor.matmul(z1[:F, :], whs[:], hsT[:],
                                         start=True, stop=False)
                        nc.tensor.matmul(z1[:F, :], weate[:], ef[:],
                                         start=False, stop=False)
                        nc.tensor.matmul(z1[:F, :], wrd[:], rdT[:],
                                         start=False, stop=False)
                        for g in range(G):
                            nc.tensor.matmul(z1[:F, g * 128:(g + 1) * 128],
                                             hwb[:], sel2[:, g, :],
                                             start=False, stop=(g == G - 1),
                                             skip_group_check=True)
                        a1 = eds.tile([F, CH], bf16, tag="a1")
                        i_a1 = nc.scalar.activation(a1[:], z1[:F, :], AF.Silu,
                                                    bias=bm1c[:])
                        if last_sx[0] is not None:
                            dep(i_a1, last_sx[0], "act-table-phase")
                        z3 = psz.tile([128, CH], f32, space="PSUM", tag="z")
                        nc.tensor.matmul(z3[:F, :], w23[:], a1[:],
                                         start=True, stop=True)
                        a3 = eds.tile([F, CH], bf16, tag="a3")
                        i_a3 = nc.scalar.activation(a3[:], z3[:F, :], AF.Silu,
                                                    bias=b3c[:])
                        if kk == nchunks - 1:
                            prev_last_a3[0] = i_a3
                        z4 = psz.tile([128, CH], f32, space="PSUM", tag="z")
                        nc.tensor.matmul(z4[:], wx2[:], a3[:],
                                         start=True, stop=True)
                        wt = eds.tile([128, CH], bf16, tag="wt")
                        nc.vector.tensor_scalar(wt[:], z4[:], bx2c[:, 0:1],
                                                CLAMP, op0=OP.add, op1=OP.min)

                        pwp = psht.tile([128, CH], bf16, space="PSUM",
                                        tag="tp")
                        for g in range(G):
                            nc.tensor.transpose(pwp[:, g * 128:(g + 1) * 128],
                                                wt[:, g * 128:(g + 1) * 128],
                                                identb[:])
                        # fwh = max(pw, -CLAMP) * 1/(1 + sqrt(rd+eps))
                        fwh = eds.tile([128, G, K], bf16, tag="fwh")
                        nc.vector.scalar_tensor_tensor(
                            fwh[:], pwp[:].rearrange("p (g k) -> p g k", g=G),
                            -CLAMP, fd[:], op0=OP.max, op1=OP.mult)
                        conth = eds.tile([128, G, XW], bf16, tag="conth")
                        for cc in range(3):
                            nc.vector.tensor_mul(
                                conth[:, :, cc * K:(cc + 1) * K],
                                rel[:, :, cc * K:(cc + 1) * K], fwh[:])
                        for g in range(G):
                            nc.tensor.matmul(upsum[:], sel[:, g, :],
                                             conth[:, g, :],
                                             start=(kk == 0 and g == 0),
                                             stop=(kk == nchunks - 1
                                                   and g == G - 1))

                    # ---------------- block output
                    oj = eds.tile([128, XW], f32, tag="oj")
                    nc.vector.tensor_add(oj[:], upsum[:], xhtb[:, 0:XW])
                    nc.sync.dma_start(out=OUT[b * 128:(b + 1) * 128, :],
                                      in_=oj[:])
                    chunk_base += nchunks

    nc.compile()
    return nc


# ---------------------------------------------------------------- emulation

def emulate_core(cfg, m, sched):
    """bf16-faithful numpy emulation of one core's program."""
    NP, NRP, NB, NCHK = cfg.NP, cfg.NRP, cfg.NB, cfg.NCHK
    NGP = cfg.NG
    nchb = list(sched)
    f32 = np.float32
    Xb = np.asarray(m["X"], f32)       # bf16 values
    Hb = np.asarray(m["H"], f32)
    ohp = np.asarray(m["ohp"], f32)
    oh = ohp.reshape(128, NCHK, NGP).transpose(1, 0, 2).reshape(NP, NGP)
    invc = m["invc"].reshape(NGP)

    ps_m = oh.T @ Xb
    Mneg = bfr(-(ps_m * invc[:, None]))
    xc_all = Xb + oh @ Mneg
    sq = bfr(xc_all ** 2)
    nsq = bfr(bfr(sq[:, :K] + sq[:, K:2 * K]) + sq[:, 2 * K:])
    nrm = bfr(np.sqrt(nsq))
    mnv = (oh.T @ nrm) * invc[:, None] + EPS_E3
    sfb = bfr((1.0 / mnv) * m["e3k"].reshape(1, K))
    sexp_all = oh @ sfb
    XP = bfr(xc_all * np.tile(sexp_all, 3))
    mu = Hb.mean(1, keepdims=True)
    var = ((Hb - mu) ** 2).mean(1, keepdims=True)
    HL = bfr((Hb - mu) / np.sqrt(var + EPS_LN))

    whs = np.asarray(m["whs"], f32)
    wht = np.asarray(m["wht"], f32)
    weate = np.asarray(m["weate"], f32)
    wrd = np.asarray(m["wrd"], f32)
    w23 = np.asarray(m["w23"], f32)
    wx2 = np.asarray(m["wx2"], f32)
    bm1 = m["bm1"].reshape(1, F)
    b3 = m["b3"].reshape(1, F)
    bx2 = m["bx2"].reshape(1, K)

    sidx = m["sidx"][:16].T.reshape(-1).astype(np.int64)
    loc = np.asarray(m["locp"], f32).T.reshape(-1).astype(np.int64)
    ef_all = np.asarray(m["EF"], f32)

    out = np.zeros((NRP, XW), f32)
    silu = lambda z: z / (1.0 + np.exp(-z))
    cch = 0
    for b in range(NB):
        upsum = np.zeros((128, XW), f32)
        XPb = XP[b * 128:(b + 1) * 128]
        HLb = HL[b * 128:(b + 1) * 128]
        hwb = bfr(HLb @ wht)
        for kk in range(nchb[b]):
            sl = slice(cch * CH, (cch + 1) * CH)
            xs = XP[sidx[sl]]
            hs = HL[sidx[sl]]
            lo = loc[sl]
            rel = bfr(xs - XPb[lo])
            sqe = bfr(rel * rel)
            rd = bfr(bfr(sqe[:, :K] + sqe[:, K:2 * K]) + sqe[:, 2 * K:])
            sxh = bfr(np.sqrt(rd + 1e-8))
            fd = bfr(1.0 / bfr(1.0 + sxh))
            ef = ef_all[:, sl].T
            z1 = hs @ whs + hwb[lo] + ef @ weate + rd @ wrd
            a1 = bfr(silu(z1 + bm1))
            z3 = a1 @ w23
            a3 = bfr(silu(z3 + b3))
            wmin = bfr(np.minimum(a3 @ wx2 + bx2, CLAMP))
            fwh = bfr(np.maximum(wmin, -CLAMP) * fd)
            conth = bfr(rel * np.tile(fwh, 3))
            np.add.at(upsum, lo, conth)
            cch += 1
        out[b * 128:(b + 1) * 128] = upsum + XPb
    return out


# ---------------------------------------------------------------- entry point

_PROGRAM_CACHE = {}


def kernel(**inputs):
    """Full-input entry: shards across 8 NeuronCores internally."""
    import sys
    for p in ("/opt/trn_rl_repo", "/root/.axon_site/_ro/trn_rl_repo"):
        if p not in sys.path:
            sys.path.append(p)
    from concourse import bass_utils

    cfg = CFG_FULL
    batch = np.asarray(inputs["batch"]).astype(np.int64)
    X = np.asarray(inputs["X"], np.float32)
    H = np.asarray(inputs["H"], np.float32)
    ei = np.asarray(inputs["edge_index"]).astype(np.int64)
    ea = np.asarray(inputs["edge_attr"], np.float32)
    te = np.asarray(inputs["te"], np.float32)

    shards, sched = build_shards(cfg, ei[0], ei[1], ea, te)
    params = make_params(cfg, *[np.asarray(inputs[k], np.float32) for k in
                         ["Wm1", "bm1", "Wm2", "bm2", "Wx1", "bx1", "Wx2",
                          "bx2", "ln_gamma", "ln_beta", "e3_weight"]])
    in_maps = [prep_core_inputs(cfg, c, shards[c], batch, X, H, params)
               for c in range(cfg.CORES)]

    key = (cfg.N, sched)
    if key not in _PROGRAM_CACHE:
        _PROGRAM_CACHE[key] = build_program(cfg, sched, cfg.CORES)
    nc = _PROGRAM_CACHE[key]

    res = bass_utils.run_bass_kernel_spmd(
        nc, in_maps, core_ids=list(range(cfg.CORES)))
    out = np.zeros((cfg.N, XW), np.float32)
    for c in range(cfg.CORES):
        out[c * cfg.NR:(c + 1) * cfg.NR] = \
            res.results[c]["OUT"][shards[c]["perm"]]
    return out.reshape(cfg.N, 3, K)


# revision 48
# speedup vs baseline: 1.0062x; 1.0062x over previous
"""EquivariantBlock Trainium kernel v2: bf16 + block-aligned chunking.

Layout / sharding:
  - 8 cores, data-parallel by target-node range (2500 nodes each, NRP=2560
    padded). Node tables replicated but ROLLED per core so the core's own
    range sits at rows [0, 2560).
  - Device preprocess (per core, replicated over all NP nodes): per-graph
    centering + E3Norm of X -> XP (bf16), LayerNorm of H -> HLN (bf16,
    gamma/beta folded into MLP weights on host), stored as one record table
    XH = [XP | HLN] ([NP, 448] bf16) in DRAM.
  - Edges sorted by target, grouped into chunks of CH=512 edges where each
    chunk's targets lie inside ONE aligned 128-node block. Per-block chunk
    counts are maxed across cores so all 8 cores share one program.
  - Per chunk: gather src records (bf16, 896B each); target side comes from
    a sequential per-block load + one-hot expansion matmuls (no tgt gather).
    rel is accumulated on the PE (identity matmul + negated-block expand).
    MLP runs feature-major at bf16 (1 cyc/row). Per-128-edge-group one-hot
    segment-sum matmuls accumulate the update in PSUM across all chunks of
    the block; OUT = upsum + XP_block written directly (no UPD table).
  - Act engine table discipline: preproc uses only sqrt-table funcs; the
    edge loop runs a sqrt phase (S) then a silu phase (M) per block, so the
    compiler inserts only 2 act-table loads per block.
"""
import numpy as np
import ml_dtypes

BF16 = ml_dtypes.bfloat16

F = 64
K = 128
XW = 3 * K          # 384
REC = XW + F        # 448
RECG = 512          # gather record (padded: 512*2B is a multiple of 256)
CH = 512            # edges per chunk
G = CH // 128       # groups per chunk
EPS_E3 = 1e-5
EPS_LN = 1e-5
CLAMP = 10.0


def bf(a):
    return np.ascontiguousarray(np.asarray(a, np.float32).astype(BF16))


def bfr(a):
    """Round to bf16, return fp32 (for emulation)."""
    return np.asarray(a, np.float32).astype(BF16).astype(np.float32)


class Cfg:
    def __init__(self, n_nodes, n_graphs, cores):
        self.N = n_nodes
        self.NG = n_graphs
        self.CORES = cores
        self.NR = n_nodes // cores        # nodes per core
        self.NRP = -(-self.NR // 128) * 128
        self.NB = self.NRP // 128         # target blocks per core
        self.NCHK = -(-n_nodes // 128)    # node chunks for preprocess
        self.NP = self.NCHK * 128


CFG_FULL = Cfg(20000, 64, 8)


# ---------------------------------------------------------------- host prep

def build_shards(cfg, src, tgt, edge_attr, te):
    """Partition edges by target block; schedule shared across cores."""
    N, NR, NB, CORES = cfg.N, cfg.NR, cfg.NB, cfg.CORES
    percore = []
    for c in range(CORES):
        em = np.where(np.minimum(tgt // NR, CORES - 1) == c)[0]
        tl0 = (tgt[em] - c * NR).astype(np.int64)
        nr_here = N - c * NR if c == CORES - 1 else NR
        deg = np.bincount(tl0, minlength=NR)
        # greedy balance: assign nodes (desc degree) to blocks, cap 128
        # nodes and minimal edge total per block -> all blocks ~N_edges/NB
        order_nodes = np.argsort(-deg, kind="stable")
        blk_of = np.zeros(NR, np.int64)
        slot_of = np.zeros(NR, np.int64)
        btot = np.zeros(NB, np.int64)
        bcnt = np.zeros(NB, np.int64)
        for v in order_nodes:
            cand = np.where(bcnt < 128)[0]
            bsel = cand[np.argmin(btot[cand])]
            blk_of[v] = bsel
            slot_of[v] = bcnt[bsel]
            btot[bsel] += deg[v]
            bcnt[bsel] += 1
        perm = blk_of * 128 + slot_of          # old local id -> new local id
        iperm = np.zeros(cfg.NRP, np.int64)
        iperm[perm] = np.arange(NR)            # new local id -> old local id
        tl = perm[tl0]
        order = np.argsort(tl, kind="stable")
        eidx = em[order]
        tl = tl[order]
        sg = src[eidx]
        blk = tl // 128
        cnt = np.bincount(blk, minlength=NB)
        percore.append(dict(eidx=eidx, tl=tl, sg=sg, cnt=cnt,
                            perm=perm, iperm=iperm))

    # shared schedule: chunks per block = max over cores
    nchb = [max(1, int(-(-max(pc["cnt"][b] for pc in percore) // CH)))
            for b in range(NB)]
    sched = tuple(nchb)
    NCH = sum(nchb)
    LE = NCH * CH

    shards = []
    for c in range(CORES):
        pc = percore[c]
        eidx, tl, sg, cnt = pc["eidx"], pc["tl"], pc["sg"], pc["cnt"]
        perm = pc["perm"]
        starts = np.concatenate([[0], np.cumsum(cnt)])
        sidx = np.zeros(LE, np.int64)
        loc = np.zeros(LE, np.int64)
        epos = np.full(LE, -1, np.int64)
        pos = 0
        for b in range(NB):
            e0, e1 = starts[b], starts[b + 1]
            ne = e1 - e0
            cap = nchb[b] * CH
            assert ne <= cap, f"block {b} core {c}: {ne} > {cap}"
            sl = slice(pos, pos + ne)
            sr = (sg[e0:e1] - c * NR) % N      # rolled row of source
            own = sr < NR
            sr = np.where(own, 0, sr + (cfg.NRP - NR))
            sr[own] = perm[((sg[e0:e1] - c * NR) % N)[own]]
            sidx[sl] = sr
            loc[sl] = tl[e0:e1] - b * 128
            epos[sl] = eidx[e0:e1]
            dl = slice(pos + ne, pos + cap)
            sidx[dl] = b * 128
            loc[dl] = 0
            pos += cap
        assert pos == LE

        ef = np.zeros((128, LE), np.float32)
        valid = epos >= 0
        ef[:F, valid] = edge_attr[epos[valid]].T
        ef[F:, valid] = te[epos[valid]].T

        def wrap16(v):
            return np.ascontiguousarray(
                np.tile(v.astype(np.int16).reshape(-1, 16).T, (8, 1)))

        shards.append(dict(
            sidx16=wrap16(sidx), perm=perm, iperm=pc["iperm"],
            locp=np.ascontiguousarray(
                loc.reshape(-1, 128).T.astype(np.float32)),
            locr=bf(loc.reshape(NCH, CH)),              # [NCH, 512]
            ef=bf(ef),
            sidx=sidx, loc=loc, epos=epos,
        ))
    return shards, sched


def make_params(cfg, Wm1, bm1, Wm2, bm2, Wx1, bx1, Wx2, bx2, ln_gamma, ln_beta,
                e3_weight):
    f = np.float32
    Wm1 = np.asarray(Wm1, f)
    g = np.asarray(ln_gamma, f).reshape(F)
    bt = np.asarray(ln_beta, f).reshape(F)
    W_ht = Wm1[0:F] * g[:, None]          # fold LN gamma into H weights
    W_hs = Wm1[F:2 * F] * g[:, None]
    b1 = (np.asarray(bm1, f).reshape(F)
          + bt @ Wm1[0:F] + bt @ Wm1[F:2 * F])   # fold LN beta
    b3 = (np.asarray(bx1, f).reshape(F)
          + np.asarray(bm2, f).reshape(F) @ np.asarray(Wx1, f))  # fold bm2
    return dict(
        whs=bf(W_hs),
        wht=bf(W_ht),
        weate=bf(np.concatenate([Wm1[2 * F:3 * F], Wm1[2 * F + K + F:]], 0)),
        wrd=bf(Wm1[3 * F:3 * F + K]),                       # [128, 64]
        w23=bf(np.asarray(Wm2, f) @ np.asarray(Wx1, f)),
        wx2=bf(Wx2),
        bm1=np.ascontiguousarray(b1.reshape(F, 1), f),
        b3=np.ascontiguousarray(b3.reshape(F, 1), f),
        bx2=np.ascontiguousarray(np.asarray(bx2, f).reshape(K, 1), f),
        e3k=np.ascontiguousarray(np.asarray(e3_weight, f).reshape(1, K), f),
    )


def make_onehots(cfg, bf_ids):
    NGP = cfg.NG
    oh = (bf_ids[:, None] == np.arange(NGP)[None, :]).astype(np.float32)
    ohp = np.ascontiguousarray(
        oh.reshape(cfg.NCHK, 128, NGP).transpose(1, 0, 2).reshape(128, -1))
    oht = np.ascontiguousarray(oh.T)
    return bf(ohp), bf(oht), NGP


def prep_core_inputs(cfg, c, shard, batch, X, H, params):
    N, NR, NP, NRP = cfg.N, cfg.NR, cfg.NP, cfg.NRP
    roll = lambda a: np.roll(a, -c * NR, axis=0)
    Xp = np.zeros((NP, XW), np.float32)
    Xp[:N] = roll(np.asarray(X, np.float32).reshape(N, XW))
    Hp = np.zeros((NP, F), np.float32)
    Hp[:N] = roll(np.asarray(H, np.float32))
    bf_ids = np.full(NP, -1.0, np.float32)
    bf_ids[:N] = roll(np.asarray(batch)).astype(np.float32)
    # permute own range into degree-balanced block order; shift the rest
    # of the rolled table up by NRP-NR rows so nothing is clobbered
    perm = shard["perm"]
    off = NRP - NR
    for arr, fill in ((Xp, 0.0), (Hp, 0.0), (bf_ids, -1.0)):
        ownnew = np.full((NRP,) + arr.shape[1:], fill, arr.dtype)
        ownnew[perm] = arr[:NR].copy()
        rest = arr[NR:NP - off].copy()
        arr[:NRP] = ownnew
        arr[NRP:NP] = rest
    ohp, oht, NGP = make_onehots(cfg, bf_ids)
    cnts = np.zeros(NGP, np.float64)
    gg, n = np.unique(np.asarray(batch), return_counts=True)
    cnts[gg.astype(np.int64)] = n
    invc_h = (1.0 / np.maximum(cnts, 1.0)).astype(np.float32).reshape(NGP, 1)
    m = dict(X=bf(Xp), H=Hp, ohp=ohp, oht=oht, invc=invc_h,
             sidx=shard["sidx16"], locp=shard["locp"], locr=shard["locr"],
             EF=shard["ef"])
    m.update(params)
    return m


# ---------------------------------------------------------------- device program

def build_program(cfg, sched, num_devices):
    import concourse.bacc as bacc
    import concourse.bass as bass
    import concourse.tile as tile
    from concourse import mybir
    from concourse.masks import make_identity
    from concourse.tile import add_dep_helper

    def dep(a, b, why):
        add_dep_helper(a.ins, b.ins, sync=True, reason=why)

    f32 = mybir.dt.float32
    bf16 = mybir.dt.bfloat16
    i16 = mybir.dt.int16
    AF = mybir.ActivationFunctionType
    OP = mybir.AluOpType

    N, NP, NCHK, NRP, NB = cfg.N, cfg.NP, cfg.NCHK, cfg.NRP, cfg.NB
    NGP = cfg.NG
    nchb = list(sched)
    NCH = sum(nchb)
    LE = NCH * CH
    NCB = max(nchb)

    nc = bacc.Bacc("TRN2", target_bir_lowering=False, debug=False,
                   num_devices=num_devices)

    def din(name, shape, dt=bf16):
        return nc.dram_tensor(name, shape, dt, kind="ExternalInput").ap()

    X = din("X", [NP, XW])
    H = din("H", [NP, F], f32)
    OHP = din("ohp", [128, NCHK * NGP])
    OHT = din("oht", [NGP, NP])
    INVC = din("invc", [NGP, 1], f32)
    SIDX = din("sidx", [128, LE // 16], i16)
    LOCP = din("locp", [128, LE // 128], f32)
    LOCR = din("locr", [NCH, CH])
    EF = din("EF", [128, LE])
    WHS = din("whs", [F, F])
    WHT = din("wht", [F, F])
    WEATE = din("weate", [2 * F, F])
    WRD = din("wrd", [K, F])
    W23 = din("w23", [F, F])
    WX2 = din("wx2", [F, K])
    BM1 = din("bm1", [F, 1], f32)
    B3 = din("b3", [F, 1], f32)
    BX2 = din("bx2", [K, 1], f32)
    E3K = din("e3k", [1, K], f32)

    OUT = nc.dram_tensor("OUT", [NRP, XW], f32, kind="ExternalOutput").ap()

    def bcast(dram_ap, parts):
        return bass.AP(tensor=dram_ap.tensor, offset=dram_ap.offset,
                       ap=[[0, parts]] + [list(p) for p in dram_ap.ap[1:]])

    def rep_mid(ap, n):
        """[P, W] -> [P, n, W] with a stride-0 middle dim."""
        aps = [list(p) for p in ap.ap]
        return bass.AP(tensor=ap.tensor, offset=ap.offset,
                       ap=aps[:-1] + [[0, n]] + aps[-1:])

    import contextlib
    with tile.TileContext(nc) as tc, \
         nc.allow_low_precision(reason="bf16 kernel, 2e-2 tolerance"), \
         contextlib.ExitStack() as ctx:
        if True:
            const = ctx.enter_context(tc.tile_pool(name="const", bufs=1))
            dramp = ctx.enter_context(tc.tile_pool(name="dram", bufs=1, space="DRAM"))

            XH = dramp.tile([NP, RECG], bf16)

            # ---- constants
            identb = const.tile([128, 128], bf16)
            make_identity(nc, identb[:])
            iotacol_i = const.tile([128, 1], mybir.dt.int32)
            nc.gpsimd.iota(iotacol_i[:], pattern=[[1, 1]], base=0,
                           channel_multiplier=1)
            iotacol = const.tile([128, 1], f32)
            nc.vector.tensor_copy(iotacol[:], iotacol_i[:])
            iotarow_i = const.tile([128, 128], mybir.dt.int32)
            nc.gpsimd.iota(iotarow_i[:], pattern=[[1, 128]], base=0,
                           channel_multiplier=0)
            iotarow = const.tile([128, 128], bf16)
            nc.vector.tensor_copy(iotarow[:], iotarow_i[:])

            whs = const.tile([F, F], bf16)
            nc.sync.dma_start(out=whs[:], in_=WHS)
            wht = const.tile([F, F], bf16)
            nc.sync.dma_start(out=wht[:], in_=WHT)
            weate = const.tile([2 * F, F], bf16)
            nc.sync.dma_start(out=weate[:], in_=WEATE)
            wrd = const.tile([K, F], bf16)
            nc.sync.dma_start(out=wrd[:], in_=WRD)
            w23 = const.tile([F, F], bf16)
            nc.sync.dma_start(out=w23[:], in_=W23)
            wx2 = const.tile([F, K], bf16)
            nc.sync.dma_start(out=wx2[:], in_=WX2)
            bm1c = const.tile([F, 1], f32)
            nc.sync.dma_start(out=bm1c[:], in_=BM1)
            b3c = const.tile([F, 1], f32)
            nc.sync.dma_start(out=b3c[:], in_=B3)
            bx2c = const.tile([K, 1], f32)
            nc.sync.dma_start(out=bx2c[:], in_=BX2)
            e3b = const.tile([NGP, K], f32)
            nc.sync.dma_start(out=e3b[:], in_=bcast(E3K, NGP))
            eps8 = const.tile([128, 1], f32)
            nc.vector.memset(eps8[:], 1e-8)
            epsln = const.tile([128, 1], f32)
            nc.vector.memset(epsln[:], EPS_LN)

            Mneg = const.tile([NGP, XW], bf16)
            sfb = const.tile([NGP, K], bf16)
            invc = const.tile([NGP, 1], f32)
            zpad8 = const.tile([128, 8, RECG - REC], bf16)
            nc.vector.memset(zpad8[:], 0.0)

            # ---- preprocessing (sqrt act-table only)
            with tc.tile_pool(name="xall", bufs=1) as xap, \
                 tc.tile_pool(name="preoh", bufs=1) as poh, \
                 tc.tile_pool(name="pre", bufs=3) as pre, \
                 tc.tile_pool(name="prep", bufs=2, space="PSUM") as pps, \
                 tc.tile_pool(name="preacc", bufs=1, space="PSUM") as pacc:

                ohp_sb = poh.tile([128, NCHK * NGP], bf16)
                nc.sync.dma_start(out=ohp_sb[:], in_=OHP)
                nc.sync.dma_start(out=invc[:], in_=INVC)

                groups = [(cb, min(4, NCHK - cb)) for cb in range(0, NCHK, 4)]

                xall = xap.tile([128, NCHK, XW], bf16)
                for cb, nb in groups:
                    nc.sync.dma_start(
                        out=xall[:, cb:cb + nb, :],
                        in_=X[cb * 128:(cb + nb) * 128, :].rearrange(
                            "(c p) w -> p c w", p=128))

                # P1: per-graph sums of X -> M_mean (negated bf16)
                ps_m = pacc.tile([NGP, XW], f32, space="PSUM")
                for ci in range(NCHK):
                    nc.tensor.matmul(ps_m[:],
                                     ohp_sb[:, ci * NGP:(ci + 1) * NGP],
                                     xall[:, ci, :], start=(ci == 0),
                                     stop=(ci == NCHK - 1))
                mmf = pre.tile([NGP, XW], f32, tag="mmf")
                nc.vector.tensor_scalar_mul(mmf[:], ps_m[:], invc[:, 0:1])
                nc.scalar.activation(Mneg[:], mmf[:], AF.Copy, scale=-1.0)

                # P2: per-graph mean vector-norm (ops batched over 4 chunks)
                ps_n = pacc.tile([NGP, K], f32, space="PSUM")
                for cb, nb in groups:
                    oht4 = pre.tile([NGP, 4, 128], bf16, tag="oht4")
                    nc.sync.dma_start(
                        out=oht4[:, :nb, :],
                        in_=OHT[:, cb * 128:(cb + nb) * 128].rearrange(
                            "g (c p) -> g c p", p=128))
                    xc4 = pps.tile([128, 4, 512], f32, space="PSUM", tag="xc",
                                   bufs=1)
                    for j in range(nb):
                        ci = cb + j
                        nc.tensor.matmul(xc4[:, j, 0:XW], oht4[:, j, :],
                                         Mneg[:], start=True, stop=False)
                        nc.tensor.matmul(xc4[:, j, 0:XW], identb[:],
                                         xall[:, ci, :],
                                         start=False, stop=True)
                    sq4 = pre.tile([128, 4, XW], bf16, tag="sq")
                    nc.scalar.activation(sq4[:, :nb, :], xc4[:, :nb, 0:XW],
                                         AF.Square)
                    nsq4 = pre.tile([128, 4, K], bf16, tag="nsq")
                    nc.vector.tensor_add(nsq4[:, :nb, :], sq4[:, :nb, 0:K],
                                         sq4[:, :nb, K:2 * K])
                    nc.vector.tensor_add(nsq4[:, :nb, :], nsq4[:, :nb, :],
                                         sq4[:, :nb, 2 * K:])
                    nrm4 = pre.tile([128, 4, K], bf16, tag="nrm")
                    nc.scalar.activation(nrm4[:, :nb, :], nsq4[:, :nb, :],
                                         AF.Sqrt)
                    for j in range(nb):
                        ci = cb + j
                        nc.tensor.matmul(ps_n[:],
                                         ohp_sb[:, ci * NGP:(ci + 1) * NGP],
                                         nrm4[:, j, :], start=(ci == 0),
                                         stop=(ci == NCHK - 1))
                mn = pre.tile([NGP, K], f32, tag="mn")
                nc.vector.tensor_scalar(mn[:], ps_n[:], invc[:, 0:1], EPS_E3,
                                        op0=OP.mult, op1=OP.add)
                rmn = pre.tile([NGP, K], f32, tag="rmn")
                nc.vector.reciprocal(rmn[:], mn[:])
                sff = pre.tile([NGP, K], f32, tag="sff")
                nc.vector.tensor_mul(sff[:], rmn[:], e3b[:])
                nc.scalar.activation(sfb[:], sff[:], AF.Copy)

                # P3: XP = (X - M[g]) * sfac[g]  -> XH[:, :XW]
                for cb, nb in groups:
                    oht4 = pre.tile([NGP, 4, 128], bf16, tag="oht4")
                    nc.sync.dma_start(
                        out=oht4[:, :nb, :],
                        in_=OHT[:, cb * 128:(cb + nb) * 128].rearrange(
                            "g (c p) -> g c p", p=128))
                    xp4 = pre.tile([128, 4, XW], bf16, tag="xp4")
                    xc4 = pps.tile([128, 4, 512], f32, space="PSUM", tag="xc",
                                   bufs=1)
                    sexp4 = pps.tile([128, 4, K], f32, space="PSUM",
                                     tag="sexp", bufs=1)
                    for j in range(nb):
                        ci = cb + j
                        nc.tensor.matmul(xc4[:, j, 0:XW], oht4[:, j, :],
                                         Mneg[:], start=True, stop=False)
                        nc.tensor.matmul(xc4[:, j, 0:XW], identb[:],
                                         xall[:, ci, :],
                                         start=False, stop=True)
                        nc.tensor.matmul(sexp4[:, j, :], oht4[:, j, :],
                                         sfb[:], start=True, stop=True)
                    sxb4 = pre.tile([128, 4, K], bf16, tag="sxb")
                    nc.vector.tensor_copy(sxb4[:, :nb, :], sexp4[:, :nb, :])
                    for j in range(nb):
                        nc.vector.scalar_tensor_tensor(
                            xp4[:, j, :].rearrange("p (d k) -> p d k", d=3),
                            xc4[:, j, 0:XW].rearrange("p (d k) -> p d k", d=3),
                            0.0, rep_mid(sxb4[:, j, :], 3),
                            op0=OP.bypass, op1=OP.mult)
                    nc.sync.dma_start(
                        out=XH[cb * 128:(cb + nb) * 128, 0:XW].rearrange(
                            "(c p) w -> p c w", p=128),
                        in_=xp4[:, :nb, :])

                # P4: HLN (no gamma/beta: folded into weights) -> XH[:, XW:]
                hgroups = [(cb, min(8, NCHK - cb)) for cb in range(0, NCHK, 8)]
                for cb, nb in hgroups:
                    h8 = pre.tile([128, 8, F], f32, tag="h8")
                    nc.sync.dma_start(
                        out=h8[:, :nb, :],
                        in_=H[cb * 128:(cb + nb) * 128, :].rearrange(
                            "(c p) w -> p c w", p=128))
                    hg8 = pre.tile([128, 8, F], bf16, tag="hg8")
                    for j in range(nb):
                        ht = h8[:, j, :]
                        st = pre.tile([128, 6], f32, tag="st")
                        nc.vector.bn_stats(out=st[:], in_=ht)
                        mv = pre.tile([128, 2], f32, tag="mv")
                        nc.vector.bn_aggr(out=mv[:], in_=st[:])
                        sd = pre.tile([128, 1], f32, tag="sd")
                        nc.scalar.activation(sd[:], mv[:, 1:2], AF.Sqrt,
                                             bias=epsln[:])
                        rs = pre.tile([128, 1], f32, tag="rs")
                        nc.vector.reciprocal(rs[:], sd[:])
                        nc.vector.tensor_scalar(hg8[:, j, :], ht, mv[:, 0:1],
                                                rs[:, 0:1],
                                                op0=OP.subtract, op1=OP.mult)
                    nc.sync.dma_start(
                        out=XH[cb * 128:(cb + nb) * 128, XW:REC].rearrange(
                            "(c p) w -> p c w", p=128),
                        in_=hg8[:, :nb, :])
                    nc.sync.dma_start(
                        out=XH[cb * 128:(cb + nb) * 128, REC:RECG].rearrange(
                            "(c p) w -> p c w", p=128),
                        in_=zpad8[:, :nb, :])

            # ---- edge loop
            with tc.tile_pool(name="edi", bufs=1) as edi, \
                 tc.tile_pool(name="blk", bufs=2) as blkp, \
                 tc.tile_pool(name="edg", bufs=3) as edg, \
                 tc.tile_pool(name="eds", bufs=2) as eds, \
                 tc.tile_pool(name="keep", bufs=NCB + 2) as keep, \
                 tc.tile_pool(name="psx", bufs=1, space="PSUM") as psx, \
                 tc.tile_pool(name="psht", bufs=1, space="PSUM") as psht, \
                 tc.tile_pool(name="psz", bufs=1, space="PSUM") as psz, \
                 tc.tile_pool(name="psu", bufs=1, space="PSUM") as psu:

                sidx_sb = edi.tile([128, LE // 16], i16)
                nc.sync.dma_start(out=sidx_sb[:], in_=SIDX)
                locp_sb = edi.tile([128, LE // 128], f32)
                nc.sync.dma_start(out=locp_sb[:], in_=LOCP)

                # preprocess XH writes land before gathers (invisible APs)
                tc.strict_bb_all_engine_barrier()

                IC = CH // 16
                gidx_reg = nc.gpsimd.alloc_register("gidx")
                nc.gpsimd.reg_mov(gidx_reg, CH)
                gath_consumers = {}
                pending = {}
                GB = 3  # xhs ring depth

                def issue_gather(cch):
                    war = gath_consumers.pop(cch - GB, None)
                    xhs = edg.tile([128, G, RECG], bf16, tag="xhs",
                                   name=f"xhs{cch}")
                    g1 = nc.gpsimd.dma_gather(
                        out_ap=xhs[:], in_ap=XH[:],
                        idxs_ap=sidx_sb[:, cch * IC:(cch + 1) * IC],
                        num_idxs=CH, num_idxs_reg=gidx_reg, elem_size=RECG,
                        single_packet=False)
                    if war:
                        for ci in war:
                            dep(g1, ci, "war-xhs")
                    pending[cch] = (xhs, g1)

                chunk_base = 0
                prev_last_a3 = [None]
                last_sx = [None]
                for b in range(NB):
                    nchunks = nchb[b]
                    c0 = chunk_base

                    xhtb = blkp.tile([128, RECG], bf16, tag="xhtb")
                    nc.sync.dma_start(out=xhtb[:],
                                      in_=XH[b * 128:(b + 1) * 128, :])

                    # HW = Hblk @ W_ht  (per block; ht-term enters z1 via sel2)
                    hbtp = psht.tile([F, CH], bf16, space="PSUM", tag="hsp")
                    nc.tensor.transpose(hbtp[:, 0:128], xhtb[:, XW:REC],
                                        identb[:])
                    hbt = blkp.tile([F, 128], bf16, tag="hbt")
                    nc.scalar.activation(hbt[:], hbtp[:, 0:128], AF.Copy)
                    hwp = psz.tile([128, CH], f32, space="PSUM", tag="z")
                    nc.tensor.matmul(hwp[:, 0:F], hbt[:], wht[:],
                                     start=True, stop=True)
                    hwb = blkp.tile([128, F], bf16, tag="hwb")
                    nc.scalar.activation(hwb[:], hwp[:, 0:F], AF.Copy)

                    upsum = psu.tile([128, XW], f32, space="PSUM", tag="u",
                                     bufs=1)

                    # ---------------- phase S (sqrt table)
                    sdata = []
                    for kk in range(nchunks):
                        cch = c0 + kk
                        if cch == 0:
                            issue_gather(0)
                            issue_gather(1)
                        if cch + 2 < NCH:
                            issue_gather(cch + 2)
                        xhs, g1 = pending.pop(cch)
                        consumers = []

                        locrep = edg.tile([128, CH], bf16, tag="locrep")
                        nc.sync.dma_start(out=locrep[:],
                                          in_=bcast(LOCR[cch:cch + 1, :], 128))
                        sel = keep.tile([128, G, 128], bf16, tag="sel")
                        sel2 = keep.tile([128, G, 128], bf16, tag="sel2")
                        rel = keep.tile([128, G, XW], bf16, tag="rel")
                        rdh = keep.tile([128, G, K], bf16, tag="rdh")
                        hsT = keep.tile([F, CH], bf16, tag="hsT")
                        rdT = keep.tile([128, CH], bf16, tag="rdT")
                        fd = keep.tile([128, G, K], bf16, tag="fd")

                        # sel[e, l] = (l == loc_e)   (Pool, per group)
                        for g in range(G):
                            nc.gpsimd.tensor_scalar(
                                sel[:, g, :], iotarow[:],
                                locp_sb[:, cch * G + g:cch * G + g + 1],
                                None, op0=OP.is_equal)
                        # sel2[l, (g,e)] = (l == loc_e)   (DVE, one op)
                        nc.vector.tensor_scalar(
                            sel2[:], locrep[:].rearrange("p (g e) -> p g e",
                                                         g=G),
                            iotacol[:, 0:1], None, op0=OP.is_equal)

                        # xpt[(g,e), :] = XP[loc_e]  (PE expand; bank-
                        # aligned 512-padded groups, two groups per substep)
                        for ss in range(G // 2):
                            xpt = psx.tile([128, 2, 512], f32, space="PSUM",
                                           tag="xpt", bufs=2)
                            for g2 in range(2):
                                g = ss * 2 + g2
                                nc.tensor.matmul(xpt[:, g2, 0:XW],
                                                 sel2[:, g, :],
                                                 xhtb[:, 0:XW],
                                                 start=True, stop=True)
                            i_rel = nc.vector.tensor_sub(
                                rel[:, ss * 2:ss * 2 + 2, :],
                                xhs[:, ss * 2:ss * 2 + 2, 0:XW],
                                xpt[:, :, 0:XW])
                            dep(i_rel, g1, "raw-xhs")
                            consumers.append(i_rel)
                        # rd = sum_c rel^2
                        sq = eds.tile([128, G, XW], bf16, tag="sq")
                        nc.scalar.activation(sq[:], rel[:], AF.Square)
                        nc.vector.tensor_add(rdh[:], sq[:, :, 0:K],
                                             sq[:, :, K:2 * K])
                        nc.vector.tensor_add(rdh[:], rdh[:], sq[:, :, 2 * K:])
                        # hsT
                        hsp = psht.tile([F, CH], bf16, space="PSUM", tag="hsp")
                        for g in range(G):
                            i_t = nc.tensor.transpose(
                                hsp[:, g * 128:(g + 1) * 128],
                                xhs[:, g, XW:REC], identb[:])
                            dep(i_t, g1, "raw-xhs-h")
                            consumers.append(i_t)
                        nc.scalar.activation(hsT[:], hsp[:], AF.Copy)
                        # rdT
                        rdp = psht.tile([128, CH], bf16, space="PSUM",
                                        tag="tp")
                        for g in range(G):
                            nc.tensor.transpose(rdp[:, g * 128:(g + 1) * 128],
                                                rdh[:, g, :], identb[:])
                        nc.scalar.activation(rdT[:], rdp[:], AF.Copy)
                        # fach = 1 / (1 + sqrt(rd + 1e-8))
                        sxh = eds.tile([128, G, K], bf16, tag="sxh")
                        i_sx = nc.scalar.activation(sxh[:], rdh[:], AF.Sqrt,
                                                    bias=eps8[:])
                        if prev_last_a3[0] is not None:
                            dep(i_sx, prev_last_a3[0], "act-table-phase")
                        last_sx[0] = i_sx
                        fdt = eds.tile([128, G, K], bf16, tag="fdt")
                        nc.vector.tensor_scalar_add(fdt[:], sxh[:], 1.0)
                        nc.vector.reciprocal(fd[:], fdt[:])
                        gath_consumers[cch] = consumers
                        sdata.append((rel, hsT, rdT, fd, sel, sel2))

                    # ---------------- phase M (silu table)
                    for kk in range(nchunks):
                        cch = c0 + kk
                        rel, hsT, rdT, fd, sel, sel2 = sdata[kk]
                        ef = edg.tile([128, CH], bf16, tag="ef")
                        nc.sync.dma_start(out=ef[:],
                                          in_=EF[:, cch * CH:(cch + 1) * CH])

                        z1 = psz.tile([128, CH], f32, space="PSUM", tag="z")
                        nc.tensor.matmul(z1[:F, :], whs[:], hsT[:],
                                         start=True, stop=False)
                        nc.tensor.matmul(z1[:F, :], weate[:], ef[:],
                                         start=False, stop=False)
                        nc.tensor.matmul(z1[:F, :], wrd[:], rdT[:],
                                         start=False, stop=False)
                        for g in range(G):
                            nc.tensor.matmul(z1[:F, g * 128:(g + 1) * 128],
                                             hwb[:], sel2[:, g, :],
                                             start=False, stop=(g == G - 1),
                                             skip_group_check=True)
                        a1 = eds.tile([F, CH], bf16, tag="a1")
                        i_a1 = nc.scalar.activation(a1[:], z1[:F, :], AF.Silu,
                                                    bias=bm1c[:])
                        if last_sx[0] is not None:
                            dep(i_a1, last_sx[0], "act-table-phase")
                        z3 = psz.tile([128, CH], f32, space="PSUM", tag="z")
                        nc.tensor.matmul(z3[:F, :], w23[:], a1[:],
                                         start=True, stop=True)
                        a3 = eds.tile([F, CH], bf16, tag="a3")
                        i_a3 = nc.scalar.activation(a3[:], z3[:F, :], AF.Silu,
                                                    bias=b3c[:])
                        if kk == nchunks - 1:
                            prev_last_a3[0] = i_a3
                        z4 = psz.tile([128, CH], f32, space="PSUM", tag="z")
                        nc.tensor.matmul(z4[:], wx2[:], a3[:],
                                         start=True, stop=True)
                        wt = eds.tile([128, CH], bf16, tag="wt")
                        nc.vector.tensor_scalar(wt[:], z4[:], bx2c[:, 0:1],
                                                CLAMP, op0=OP.add, op1=OP.min)

                        pwp = psht.tile([128, CH], bf16, space="PSUM",
                                        tag="tp")
                        for g in range(G):
                            nc.tensor.transpose(pwp[:, g * 128:(g + 1) * 128],
                                                wt[:, g * 128:(g + 1) * 128],
                                                identb[:])
                        # fwh = max(pw, -CLAMP) * 1/(1 + sqrt(rd+eps))
                        fwh = eds.tile([128, G, K], bf16, tag="fwh")
                        nc.vector.scalar_tensor_tensor(
                            fwh[:], pwp[:].rearrange("p (g k) -> p g k", g=G),
                            -CLAMP, fd[:], op0=OP.max, op1=OP.mult)
                        conth = eds.tile([128, G, XW], bf16, tag="conth")
                        for cc in range(3):
                            nc.vector.tensor_mul(
                                conth[:, :, cc * K:(cc + 1) * K],
                                rel[:, :, cc * K:(cc + 1) * K], fwh[:])
                        for g in range(G):
                            nc.tensor.matmul(upsum[:], sel[:, g, :],
                                             conth[:, g, :],
                                             start=(kk == 0 and g == 0),
                                             stop=(kk == nchunks - 1
                                                   and g == G - 1))

                    # ---------------- block output
                    oj = eds.tile([128, XW], f32, tag="oj")
                    nc.vector.tensor_add(oj[:], upsum[:], xhtb[:, 0:XW])
                    nc.sync.dma_start(out=OUT[b * 128:(b + 1) * 128, :],
                                      in_=oj[:])
                    chunk_base += nchunks

    nc.compile()
    return nc


# ---------------------------------------------------------------- emulation

def emulate_core(cfg, m, sched):
    """bf16-faithful numpy emulation of one core's program."""
    NP, NRP, NB, NCHK = cfg.NP, cfg.NRP, cfg.NB, cfg.NCHK
    NGP = cfg.NG
    nchb = list(sched)
    f32 = np.float32
    Xb = np.asarray(m["X"], f32)       # bf16 values
    Hb = np.asarray(m["H"], f32)
    ohp = np.asarray(m["ohp"], f32)
    oh = ohp.reshape(128, NCHK, NGP).transpose(1, 0, 2).reshape(NP, NGP)
    invc = m["invc"].reshape(NGP)

    ps_m = oh.T @ Xb
    Mneg = bfr(-(ps_m * invc[:, None]))
    xc_all = Xb + oh @ Mneg
    sq = bfr(xc_all ** 2)
    nsq = bfr(bfr(sq[:, :K] + sq[:, K:2 * K]) + sq[:, 2 * K:])
    nrm = bfr(np.sqrt(nsq))
    mnv = (oh.T @ nrm) * invc[:, None] + EPS_E3
    sfb = bfr((1.0 / mnv) * m["e3k"].reshape(1, K))
    sexp_all = oh @ sfb
    XP = bfr(xc_all * np.tile(sexp_all, 3))
    mu = Hb.mean(1, keepdims=True)
    var = ((Hb - mu) ** 2).mean(1, keepdims=True)
    HL = bfr((Hb - mu) / np.sqrt(var + EPS_LN))

    whs = np.asarray(m["whs"], f32)
    wht = np.asarray(m["wht"], f32)
    weate = np.asarray(m["weate"], f32)
    wrd = np.asarray(m["wrd"], f32)
    w23 = np.asarray(m["w23"], f32)
    wx2 = np.asarray(m["wx2"], f32)
    bm1 = m["bm1"].reshape(1, F)
    b3 = m["b3"].reshape(1, F)
    bx2 = m["bx2"].reshape(1, K)

    sidx = m["sidx"][:16].T.reshape(-1).astype(np.int64)
    loc = np.asarray(m["locp"], f32).T.reshape(-1).astype(np.int64)
    ef_all = np.asarray(m["EF"], f32)

    out = np.zeros((NRP, XW), f32)
    silu = lambda z: z / (1.0 + np.exp(-z))
    cch = 0
    for b in range(NB):
        upsum = np.zeros((128, XW), f32)
        XPb = XP[b * 128:(b + 1) * 128]
        HLb = HL[b * 128:(b + 1) * 128]
        hwb = bfr(HLb @ wht)
        for kk in range(nchb[b]):
            sl = slice(cch * CH, (cch + 1) * CH)
            xs = XP[sidx[sl]]
            hs = HL[sidx[sl]]
            lo = loc[sl]
            rel = bfr(xs - XPb[lo])
            sqe = bfr(rel * rel)
            rd = bfr(bfr(sqe[:, :K] + sqe[:, K:2 * K]) + sqe[:, 2 * K:])
            sxh = bfr(np.sqrt(rd + 1e-8))
            fd = bfr(1.0 / bfr(1.0 + sxh))
            ef = ef_all[:, sl].T
            z1 = hs @ whs + hwb[lo] + ef @ weate + rd @ wrd
            a1 = bfr(silu(z1 + bm1))
            z3 = a1 @ w23
            a3 = bfr(silu(z3 + b3))
            wmin = bfr(np.minimum(a3 @ wx2 + bx2, CLAMP))
            fwh = bfr(np.maximum(wmin, -CLAMP) * fd)
            conth = bfr(rel * np.tile(fwh, 3))
            np.add.at(upsum, lo, conth)
            cch += 1
        out[b * 128:(b + 1) * 128] = upsum + XPb
    return out


# ---------------------------------------------------------------- entry point

_PROGRAM_CACHE = {}


def kernel(**inputs):
    """Full-input entry: shards across 8 NeuronCores internally."""
    import sys
    for p in ("/opt/trn_rl_repo", "/root/.axon_site/_ro/trn_rl_repo"):
        if p not in sys.path:
            sys.path.append(p)
    from concourse import bass_utils

    cfg = CFG_FULL
    batch = np.asarray(inputs["batch"]).astype(np.int64)
    X = np.asarray(inputs["X"], np.float32)
    H = np.asarray(inputs["H"], np.float32)
    ei = np.asarray(inputs["edge_index"]).astype(np.int64)
    ea = np.asarray(inputs["edge_attr"], np.float32)
    te = np.asarray(inputs["te"], np.float32)

    shards, sched = build_shards(cfg, ei[0], ei[1], ea, te)
    params = make_params(cfg, *[np.asarray(inputs[k], np.float32) for k in
                         ["Wm1", "bm1", "Wm2", "bm2", "Wx1", "bx1", "Wx2",
                          "bx2", "ln_gamma", "ln_beta", "e3_weight"]])
    in_maps = [prep_core_inputs(cfg, c, shards[c], batch, X, H, params)
               for c in range(cfg.CORES)]

    key = (cfg.N, sched)
    if key not in _PROGRAM_CACHE:
        _PROGRAM_CACHE[key] = build_program(cfg, sched, cfg.CORES)
    nc = _PROGRAM_CACHE[key]

    res = bass_utils.run_bass_kernel_spmd(
        nc, in_maps, core_ids=list(range(cfg.CORES)))
    out = np.zeros((cfg.N, XW), np.float32)
    for c in range(cfg.CORES):
        out[c * cfg.NR:(c + 1) * cfg.NR] = \
            res.results[c]["OUT"][shards[c]["perm"]]
    return out.reshape(cfg.N, 3, K)


# revision 55
# speedup vs baseline: 1.0329x; 1.0266x over previous
"""EquivariantBlock Trainium kernel v2: bf16 + block-aligned chunking.

Layout / sharding:
  - 8 cores, data-parallel by target-node range (2500 nodes each, NRP=2560
    padded). Node tables replicated but ROLLED per core so the core's own
    range sits at rows [0, 2560).
  - Device preprocess (per core, replicated over all NP nodes): per-graph
    centering + E3Norm of X -> XP (bf16), LayerNorm of H -> HLN (bf16,
    gamma/beta folded into MLP weights on host), stored as one record table
    XH = [XP | HLN] ([NP, 448] bf16) in DRAM.
  - Edges sorted by target, grouped into chunks of CH=512 edges where each
    chunk's targets lie inside ONE aligned 128-node block. Per-block chunk
    counts are maxed across cores so all 8 cores share one program.
  - Per chunk: gather src records (bf16, 896B each); target side comes from
    a sequential per-block load + one-hot expansion matmuls (no tgt gather).
    rel is accumulated on the PE (identity matmul + negated-block expand).
    MLP runs feature-major at bf16 (1 cyc/row). Per-128-edge-group one-hot
    segment-sum matmuls accumulate the update in PSUM across all chunks of
    the block; OUT = upsum + XP_block written directly (no UPD table).
  - Act engine table discipline: preproc uses only sqrt-table funcs; the
    edge loop runs a sqrt phase (S) then a silu phase (M) per block, so the
    compiler inserts only 2 act-table loads per block.
"""
import numpy as np
import ml_dtypes

BF16 = ml_dtypes.bfloat16

F = 64
K = 128
XW = 3 * K          # 384
REC = XW + F        # 448
RECG = 512          # gather record (padded: 512*2B is a multiple of 256)
CH = 512            # edges per chunk
G = CH // 128       # groups per chunk
EPS_E3 = 1e-5
EPS_LN = 1e-5
CLAMP = 10.0


def bf(a):
    return np.ascontiguousarray(np.asarray(a, np.float32).astype(BF16))


def bfr(a):
    """Round to bf16, return fp32 (for emulation)."""
    return np.asarray(a, np.float32).astype(BF16).astype(np.float32)


class Cfg:
    def __init__(self, n_nodes, n_graphs, cores):
        self.N = n_nodes
        self.NG = n_graphs
        self.CORES = cores
        self.NR = n_nodes // cores        # nodes per core
        self.NRP = -(-self.NR // 128) * 128
        self.NB = self.NRP // 128         # target blocks per core
        self.NCHK = -(-n_nodes // 128)    # node chunks for preprocess
        self.NP = self.NCHK * 128


CFG_FULL = Cfg(20000, 64, 8)


# ---------------------------------------------------------------- host prep

def build_shards(cfg, src, tgt, edge_attr, te):
    """Partition edges by target block; schedule shared across cores."""
    N, NR, NB, CORES = cfg.N, cfg.NR, cfg.NB, cfg.CORES
    percore = []
    for c in range(CORES):
        em = np.where(np.minimum(tgt // NR, CORES - 1) == c)[0]
        tl0 = (tgt[em] - c * NR).astype(np.int64)
        nr_here = N - c * NR if c == CORES - 1 else NR
        deg = np.bincount(tl0, minlength=NR)
        # greedy balance: assign nodes (desc degree) to blocks, cap 128
        # nodes and minimal edge total per block -> all blocks ~N_edges/NB
        order_nodes = np.argsort(-deg, kind="stable")
        blk_of = np.zeros(NR, np.int64)
        slot_of = np.zeros(NR, np.int64)
        btot = np.zeros(NB, np.int64)
        bcnt = np.zeros(NB, np.int64)
        for v in order_nodes:
            cand = np.where(bcnt < 128)[0]
            bsel = cand[np.argmin(btot[cand])]
            blk_of[v] = bsel
            slot_of[v] = bcnt[bsel]
            btot[bsel] += deg[v]
            bcnt[bsel] += 1
        perm = blk_of * 128 + slot_of          # old local id -> new local id
        iperm = np.zeros(cfg.NRP, np.int64)
        iperm[perm] = np.arange(NR)            # new local id -> old local id
        tl = perm[tl0]
        order = np.argsort(tl, kind="stable")
        eidx = em[order]
        tl = tl[order]
        sg = src[eidx]
        blk = tl // 128
        cnt = np.bincount(blk, minlength=NB)
        percore.append(dict(eidx=eidx, tl=tl, sg=sg, cnt=cnt,
                            perm=perm, iperm=iperm))

    # shared schedule: chunks per block = max over cores
    nchb = [max(1, int(-(-max(pc["cnt"][b] for pc in percore) // CH)))
            for b in range(NB)]
    sched = tuple(nchb)
    NCH = sum(nchb)
    LE = NCH * CH

    shards = []
    for c in range(CORES):
        pc = percore[c]
        eidx, tl, sg, cnt = pc["eidx"], pc["tl"], pc["sg"], pc["cnt"]
        perm = pc["perm"]
        starts = np.concatenate([[0], np.cumsum(cnt)])
        sidx = np.zeros(LE, np.int64)
        loc = np.zeros(LE, np.int64)
        epos = np.full(LE, -1, np.int64)
        pos = 0
        for b in range(NB):
            e0, e1 = starts[b], starts[b + 1]
            ne = e1 - e0
            cap = nchb[b] * CH
            assert ne <= cap, f"block {b} core {c}: {ne} > {cap}"
            sl = slice(pos, pos + ne)
            sr = (sg[e0:e1] - c * NR) % N      # rolled row of source
            own = sr < NR
            sr = np.where(own, 0, sr + (cfg.NRP - NR))
            sr[own] = perm[((sg[e0:e1] - c * NR) % N)[own]]
            sidx[sl] = sr
            loc[sl] = tl[e0:e1] - b * 128
            epos[sl] = eidx[e0:e1]
            dl = slice(pos + ne, pos + cap)
            sidx[dl] = b * 128
            loc[dl] = 0
            pos += cap
        assert pos == LE

        ef = np.zeros((128, LE), np.float32)
        valid = epos >= 0
        ef[:F, valid] = edge_attr[epos[valid]].T
        ef[F:, valid] = te[epos[valid]].T

        def wrap16(v):
            return np.ascontiguousarray(
                np.tile(v.astype(np.int16).reshape(-1, 16).T, (8, 1)))

        shards.append(dict(
            sidx16=wrap16(sidx), perm=perm, iperm=pc["iperm"],
            locp=np.ascontiguousarray(
                loc.reshape(-1, 128).T.astype(np.float32)),
            locr=bf(loc.reshape(NCH, CH)),              # [NCH, 512]
            ef=bf(ef),
            sidx=sidx, loc=loc, epos=epos,
        ))
    return shards, sched


def make_params(cfg, Wm1, bm1, Wm2, bm2, Wx1, bx1, Wx2, bx2, ln_gamma, ln_beta,
                e3_weight):
    f = np.float32
    Wm1 = np.asarray(Wm1, f)
    g = np.asarray(ln_gamma, f).reshape(F)
    bt = np.asarray(ln_beta, f).reshape(F)
    W_ht = Wm1[0:F] * g[:, None]          # fold LN gamma into H weights
    W_hs = Wm1[F:2 * F] * g[:, None]
    b1 = (np.asarray(bm1, f).reshape(F)
          + bt @ Wm1[0:F] + bt @ Wm1[F:2 * F])   # fold LN beta
    b3 = (np.asarray(bx1, f).reshape(F)
          + np.asarray(bm2, f).reshape(F) @ np.asarray(Wx1, f))  # fold bm2
    return dict(
        whs=bf(W_hs),
        wht=bf(W_ht),
        weate=bf(np.concatenate([Wm1[2 * F:3 * F], Wm1[2 * F + K + F:]], 0)),
        wrd=bf(Wm1[3 * F:3 * F + K]),                       # [128, 64]
        w23=bf(np.asarray(Wm2, f) @ np.asarray(Wx1, f)),
        wx2=bf(Wx2),
        bm1=np.ascontiguousarray(b1.reshape(F, 1), f),
        b3=np.ascontiguousarray(b3.reshape(F, 1), f),
        bx2=np.ascontiguousarray(np.asarray(bx2, f).reshape(K, 1), f),
        e3k=np.ascontiguousarray(np.asarray(e3_weight, f).reshape(1, K), f),
    )


def make_onehots(cfg, bf_ids):
    NGP = cfg.NG
    oh = (bf_ids[:, None] == np.arange(NGP)[None, :]).astype(np.float32)
    ohp = np.ascontiguousarray(
        oh.reshape(cfg.NCHK, 128, NGP).transpose(1, 0, 2).reshape(128, -1))
    oht = np.ascontiguousarray(oh.T)
    return bf(ohp), bf(oht), NGP


def prep_core_inputs(cfg, c, shard, batch, X, H, params):
    N, NR, NP, NRP = cfg.N, cfg.NR, cfg.NP, cfg.NRP
    roll = lambda a: np.roll(a, -c * NR, axis=0)
    Xp = np.zeros((NP, XW), np.float32)
    Xp[:N] = roll(np.asarray(X, np.float32).reshape(N, XW))
    Hp = np.zeros((NP, F), np.float32)
    Hp[:N] = roll(np.asarray(H, np.float32))
    bf_ids = np.full(NP, -1.0, np.float32)
    bf_ids[:N] = roll(np.asarray(batch)).astype(np.float32)
    # permute own range into degree-balanced block order; shift the rest
    # of the rolled table up by NRP-NR rows so nothing is clobbered
    perm = shard["perm"]
    off = NRP - NR
    for arr, fill in ((Xp, 0.0), (Hp, 0.0), (bf_ids, -1.0)):
        ownnew = np.full((NRP,) + arr.shape[1:], fill, arr.dtype)
        ownnew[perm] = arr[:NR].copy()
        rest = arr[NR:NP - off].copy()
        arr[:NRP] = ownnew
        arr[NRP:NP] = rest
    ohp, oht, NGP = make_onehots(cfg, bf_ids)
    cnts = np.zeros(NGP, np.float64)
    gg, n = np.unique(np.asarray(batch), return_counts=True)
    cnts[gg.astype(np.int64)] = n
    invc_h = (1.0 / np.maximum(cnts, 1.0)).astype(np.float32).reshape(NGP, 1)
    m = dict(X=bf(Xp), H=Hp, ohp=ohp, oht=oht, invc=invc_h,
             sidx=shard["sidx16"], locp=shard["locp"], locr=shard["locr"],
             EF=shard["ef"])
    m.update(params)
    return m


# ---------------------------------------------------------------- device program

def build_program(cfg, sched, num_devices):
    import concourse.bacc as bacc
    import concourse.bass as bass
    import concourse.tile as tile
    from concourse import mybir
    from concourse.masks import make_identity
    from concourse.tile import add_dep_helper

    def dep(a, b, why):
        add_dep_helper(a.ins, b.ins, sync=True, reason=why)

    f32 = mybir.dt.float32
    bf16 = mybir.dt.bfloat16
    i16 = mybir.dt.int16
    AF = mybir.ActivationFunctionType
    OP = mybir.AluOpType

    N, NP, NCHK, NRP, NB = cfg.N, cfg.NP, cfg.NCHK, cfg.NRP, cfg.NB
    NGP = cfg.NG
    nchb = list(sched)
    NCH = sum(nchb)
    LE = NCH * CH
    NCB = max(nchb)

    nc = bacc.Bacc("TRN2", target_bir_lowering=False, debug=False,
                   num_devices=num_devices)

    def din(name, shape, dt=bf16):
        return nc.dram_tensor(name, shape, dt, kind="ExternalInput").ap()

    X = din("X", [NP, XW])
    H = din("H", [NP, F], f32)
    OHP = din("ohp", [128, NCHK * NGP])
    OHT = din("oht", [NGP, NP])
    INVC = din("invc", [NGP, 1], f32)
    SIDX = din("sidx", [128, LE // 16], i16)
    LOCP = din("locp", [128, LE // 128], f32)
    LOCR = din("locr", [NCH, CH])
    EF = din("EF", [128, LE])
    WHS = din("whs", [F, F])
    WHT = din("wht", [F, F])
    WEATE = din("weate", [2 * F, F])
    WRD = din("wrd", [K, F])
    W23 = din("w23", [F, F])
    WX2 = din("wx2", [F, K])
    BM1 = din("bm1", [F, 1], f32)
    B3 = din("b3", [F, 1], f32)
    BX2 = din("bx2", [K, 1], f32)
    E3K = din("e3k", [1, K], f32)

    OUT = nc.dram_tensor("OUT", [NRP, XW], f32, kind="ExternalOutput").ap()

    def bcast(dram_ap, parts):
        return bass.AP(tensor=dram_ap.tensor, offset=dram_ap.offset,
                       ap=[[0, parts]] + [list(p) for p in dram_ap.ap[1:]])

    def rep_mid(ap, n):
        """[P, W] -> [P, n, W] with a stride-0 middle dim."""
        aps = [list(p) for p in ap.ap]
        return bass.AP(tensor=ap.tensor, offset=ap.offset,
                       ap=aps[:-1] + [[0, n]] + aps[-1:])

    import contextlib
    with tile.TileContext(nc) as tc, \
         nc.allow_low_precision(reason="bf16 kernel, 2e-2 tolerance"), \
         contextlib.ExitStack() as ctx:
        if True:
            const = ctx.enter_context(tc.tile_pool(name="const", bufs=1))
            dramp = ctx.enter_context(tc.tile_pool(name="dram", bufs=1, space="DRAM"))

            XH = dramp.tile([NP, RECG], bf16)

            # ---- constants
            identb = const.tile([128, 128], bf16)
            make_identity(nc, identb[:])
            iotacol_i = const.tile([128, 1], mybir.dt.int32)
            nc.gpsimd.iota(iotacol_i[:], pattern=[[1, 1]], base=0,
                           channel_multiplier=1)
            iotacol = const.tile([128, 1], f32)
            nc.vector.tensor_copy(iotacol[:], iotacol_i[:])
            iotarow_i = const.tile([128, 128], mybir.dt.int32)
            nc.gpsimd.iota(iotarow_i[:], pattern=[[1, 128]], base=0,
                           channel_multiplier=0)
            iotarow = const.tile([128, 128], bf16)
            nc.vector.tensor_copy(iotarow[:], iotarow_i[:])

            whs = const.tile([F, F], bf16)
            nc.sync.dma_start(out=whs[:], in_=WHS)
            wht = const.tile([F, F], bf16)
            nc.sync.dma_start(out=wht[:], in_=WHT)
            weate = const.tile([2 * F, F], bf16)
            nc.sync.dma_start(out=weate[:], in_=WEATE)
            wrd = const.tile([K, F], bf16)
            nc.sync.dma_start(out=wrd[:], in_=WRD)
            w23 = const.tile([F, F], bf16)
            nc.sync.dma_start(out=w23[:], in_=W23)
            wx2 = const.tile([F, K], bf16)
            nc.sync.dma_start(out=wx2[:], in_=WX2)
            bm1c = const.tile([F, 1], f32)
            nc.sync.dma_start(out=bm1c[:], in_=BM1)
            b3c = const.tile([F, 1], f32)
            nc.sync.dma_start(out=b3c[:], in_=B3)
            bx2c = const.tile([K, 1], f32)
            nc.sync.dma_start(out=bx2c[:], in_=BX2)
            e3b = const.tile([NGP, K], f32)
            nc.sync.dma_start(out=e3b[:], in_=bcast(E3K, NGP))
            eps8 = const.tile([128, 1], f32)
            nc.vector.memset(eps8[:], 1e-8)
            epsln = const.tile([128, 1], f32)
            nc.vector.memset(epsln[:], EPS_LN)

            Mneg = const.tile([NGP, XW], bf16)
            sfb = const.tile([NGP, K], bf16)
            invc = const.tile([NGP, 1], f32)
            zpad8 = const.tile([128, 8, RECG - REC], bf16)
            nc.vector.memset(zpad8[:], 0.0)

            # ---- preprocessing (sqrt act-table only)
            with tc.tile_pool(name="xall", bufs=1) as xap, \
                 tc.tile_pool(name="preoh", bufs=1) as poh, \
                 tc.tile_pool(name="pre", bufs=3) as pre, \
                 tc.tile_pool(name="prep", bufs=2, space="PSUM") as pps, \
                 tc.tile_pool(name="preacc", bufs=1, space="PSUM") as pacc:

                ohp_sb = poh.tile([128, NCHK * NGP], bf16)
                nc.sync.dma_start(out=ohp_sb[:], in_=OHP)
                nc.sync.dma_start(out=invc[:], in_=INVC)

                groups = [(cb, min(4, NCHK - cb)) for cb in range(0, NCHK, 4)]

                xall = xap.tile([128, NCHK, XW], bf16)
                for cb, nb in groups:
                    nc.sync.dma_start(
                        out=xall[:, cb:cb + nb, :],
                        in_=X[cb * 128:(cb + nb) * 128, :].rearrange(
                            "(c p) w -> p c w", p=128))

                # P1: per-graph sums of X -> M_mean (negated bf16)
                ps_m = pacc.tile([NGP, XW], f32, space="PSUM")
                for ci in range(NCHK):
                    nc.tensor.matmul(ps_m[:],
                                     ohp_sb[:, ci * NGP:(ci + 1) * NGP],
                                     xall[:, ci, :], start=(ci == 0),
                                     stop=(ci == NCHK - 1))
                mmf = pre.tile([NGP, XW], f32, tag="mmf")
                nc.vector.tensor_scalar_mul(mmf[:], ps_m[:], invc[:, 0:1])
                nc.scalar.activation(Mneg[:], mmf[:], AF.Copy, scale=-1.0)

                # P2: per-graph mean vector-norm (ops batched over 4 chunks)
                ps_n = pacc.tile([NGP, K], f32, space="PSUM")
                for cb, nb in groups:
                    oht4 = pre.tile([NGP, 4, 128], bf16, tag="oht4")
                    nc.sync.dma_start(
                        out=oht4[:, :nb, :],
                        in_=OHT[:, cb * 128:(cb + nb) * 128].rearrange(
                            "g (c p) -> g c p", p=128))
                    xc4 = pps.tile([128, 4, 512], f32, space="PSUM", tag="xc",
                                   bufs=1)
                    for j in range(nb):
                        ci = cb + j
                        nc.tensor.matmul(xc4[:, j, 0:XW], oht4[:, j, :],
                                         Mneg[:], start=True, stop=False)
                        nc.tensor.matmul(xc4[:, j, 0:XW], identb[:],
                                         xall[:, ci, :],
                                         start=False, stop=True)
                    sq4 = pre.tile([128, 4, XW], bf16, tag="sq")
                    nc.scalar.activation(sq4[:, :nb, :], xc4[:, :nb, 0:XW],
                                         AF.Square)
                    nsq4 = pre.tile([128, 4, K], bf16, tag="nsq")
                    nc.vector.tensor_add(nsq4[:, :nb, :], sq4[:, :nb, 0:K],
                                         sq4[:, :nb, K:2 * K])
                    nc.vector.tensor_add(nsq4[:, :nb, :], nsq4[:, :nb, :],
                                         sq4[:, :nb, 2 * K:])
                    nrm4 = pre.tile([128, 4, K], bf16, tag="nrm")
                    nc.scalar.activation(nrm4[:, :nb, :], nsq4[:, :nb, :],
                                         AF.Sqrt)
                    for j in range(nb):
                        ci = cb + j
                        nc.tensor.matmul(ps_n[:],
                                         ohp_sb[:, ci * NGP:(ci + 1) * NGP],
                                         nrm4[:, j, :], start=(ci == 0),
                                         stop=(ci == NCHK - 1))
                mn = pre.tile([NGP, K], f32, tag="mn")
                nc.vector.tensor_scalar(mn[:], ps_n[:], invc[:, 0:1], EPS_E3,
                                        op0=OP.mult, op1=OP.add)
                rmn = pre.tile([NGP, K], f32, tag="rmn")
                nc.vector.reciprocal(rmn[:], mn[:])
                sff = pre.tile([NGP, K], f32, tag="sff")
                nc.vector.tensor_mul(sff[:], rmn[:], e3b[:])
                nc.scalar.activation(sfb[:], sff[:], AF.Copy)

                # P3: XP = (X - M[g]) * sfac[g]  -> XH[:, :XW]
                for cb, nb in groups:
                    oht4 = pre.tile([NGP, 4, 128], bf16, tag="oht4")
                    nc.sync.dma_start(
                        out=oht4[:, :nb, :],
                        in_=OHT[:, cb * 128:(cb + nb) * 128].rearrange(
                            "g (c p) -> g c p", p=128))
                    xp4 = pre.tile([128, 4, XW], bf16, tag="xp4")
                    xc4 = pps.tile([128, 4, 512], f32, space="PSUM", tag="xc",
                                   bufs=1)
                    sexp4 = pps.tile([128, 4, K], f32, space="PSUM",
                                     tag="sexp", bufs=1)
                    for j in range(nb):
                        ci = cb + j
                        nc.tensor.matmul(xc4[:, j, 0:XW], oht4[:, j, :],
                                         Mneg[:], start=True, stop=False)
                        nc.tensor.matmul(xc4[:, j, 0:XW], identb[:],
                                         xall[:, ci, :],
                                         start=False, stop=True)
                        nc.tensor.matmul(sexp4[:, j, :], oht4[:, j, :],
                                         sfb[:], start=True, stop=True)
                    sxb4 = pre.tile([128, 4, K], bf16, tag="sxb")
                    nc.vector.tensor_copy(sxb4[:, :nb, :], sexp4[:, :nb, :])
                    for j in range(nb):
                        nc.vector.scalar_tensor_tensor(
                            xp4[:, j, :].rearrange("p (d k) -> p d k", d=3),
                            xc4[:, j, 0:XW].rearrange("p (d k) -> p d k", d=3),
                            0.0, rep_mid(sxb4[:, j, :], 3),
                            op0=OP.bypass, op1=OP.mult)
                    nc.sync.dma_start(
                        out=XH[cb * 128:(cb + nb) * 128, 0:XW].rearrange(
                            "(c p) w -> p c w", p=128),
                        in_=xp4[:, :nb, :])

                # P4: HLN (no gamma/beta: folded into weights) -> XH[:, XW:]
                hgroups = [(cb, min(8, NCHK - cb)) for cb in range(0, NCHK, 8)]
                for cb, nb in hgroups:
                    h8 = pre.tile([128, 8, F], f32, tag="h8")
                    nc.sync.dma_start(
                        out=h8[:, :nb, :],
                        in_=H[cb * 128:(cb + nb) * 128, :].rearrange(
                            "(c p) w -> p c w", p=128))
                    hg8 = pre.tile([128, 8, F], bf16, tag="hg8")
                    for j in range(nb):
                        ht = h8[:, j, :]
                        st = pre.tile([128, 6], f32, tag="st")
                        nc.vector.bn_stats(out=st[:], in_=ht)
                        mv = pre.tile([128, 2], f32, tag="mv")
                        nc.vector.bn_aggr(out=mv[:], in_=st[:])
                        sd = pre.tile([128, 1], f32, tag="sd")
                        nc.scalar.activation(sd[:], mv[:, 1:2], AF.Sqrt,
                                             bias=epsln[:])
                        rs = pre.tile([128, 1], f32, tag="rs")
                        nc.vector.reciprocal(rs[:], sd[:])
                        nc.vector.tensor_scalar(hg8[:, j, :], ht, mv[:, 0:1],
                                                rs[:, 0:1],
                                                op0=OP.subtract, op1=OP.mult)
                    nc.sync.dma_start(
                        out=XH[cb * 128:(cb + nb) * 128, XW:REC].rearrange(
                            "(c p) w -> p c w", p=128),
                        in_=hg8[:, :nb, :])
                    nc.sync.dma_start(
                        out=XH[cb * 128:(cb + nb) * 128, REC:RECG].rearrange(
                            "(c p) w -> p c w", p=128),
                        in_=zpad8[:, :nb, :])

            # ---- edge loop
            with tc.tile_pool(name="edi", bufs=1) as edi, \
                 tc.tile_pool(name="blk", bufs=2) as blkp, \
                 tc.tile_pool(name="edg", bufs=3) as edg, \
                 tc.tile_pool(name="eds", bufs=2) as eds, \
                 tc.tile_pool(name="keep", bufs=NCB + 5) as keep, \
                 tc.tile_pool(name="psx", bufs=1, space="PSUM") as psx, \
                 tc.tile_pool(name="psht", bufs=1, space="PSUM") as psht, \
                 tc.tile_pool(name="psz", bufs=1, space="PSUM") as psz, \
                 tc.tile_pool(name="psu", bufs=1, space="PSUM") as psu:

                sidx_sb = edi.tile([128, LE // 16], i16)
                nc.sync.dma_start(out=sidx_sb[:], in_=SIDX)
                locp_sb = edi.tile([128, LE // 128], f32)
                nc.sync.dma_start(out=locp_sb[:], in_=LOCP)

                # preprocess XH writes land before gathers (invisible APs)
                tc.strict_bb_all_engine_barrier()

                IC = CH // 16
                gidx_reg = nc.gpsimd.alloc_register("gidx")
                nc.gpsimd.reg_mov(gidx_reg, CH)
                gath_consumers = {}
                pending = {}
                GB = 3  # xhs ring depth

                def issue_gather(cch):
                    war = gath_consumers.pop(cch - GB, None)
                    xhs = edg.tile([128, G, RECG], bf16, tag="xhs",
                                   name=f"xhs{cch}")
                    g1 = nc.gpsimd.dma_gather(
                        out_ap=xhs[:], in_ap=XH[:],
                        idxs_ap=sidx_sb[:, cch * IC:(cch + 1) * IC],
                        num_idxs=CH, num_idxs_reg=gidx_reg, elem_size=RECG,
                        single_packet=False)
                    if war:
                        for ci in war:
                            dep(g1, ci, "war-xhs")
                    pending[cch] = (xhs, g1)

                chunk_base = 0
                prev_last_a3 = [None]
                last_sx = [None]
                for b in range(NB):
                    nchunks = nchb[b]
                    c0 = chunk_base

                    xhtb = blkp.tile([128, RECG], bf16, tag="xhtb")
                    nc.sync.dma_start(out=xhtb[:],
                                      in_=XH[b * 128:(b + 1) * 128, :])

                    # HW = Hblk @ W_ht  (per block; ht-term enters z1 via sel2)
                    hbtp = psht.tile([F, CH], bf16, space="PSUM", tag="hsp")
                    nc.tensor.transpose(hbtp[:, 0:128], xhtb[:, XW:REC],
                                        identb[:])
                    hbt = blkp.tile([F, 128], bf16, tag="hbt")
                    nc.scalar.activation(hbt[:], hbtp[:, 0:128], AF.Copy)
                    hwp = psz.tile([128, CH], f32, space="PSUM", tag="z")
                    nc.tensor.matmul(hwp[:, 0:F], hbt[:], wht[:],
                                     start=True, stop=True)
                    hwb = blkp.tile([128, F], bf16, tag="hwb")
                    nc.scalar.activation(hwb[:], hwp[:, 0:F], AF.Copy)

                    upsum = psu.tile([128, XW], f32, space="PSUM", tag="u",
                                     bufs=1)

                    # ---------------- phase S (sqrt table)
                    sdata = []
                    for kk in range(nchunks):
                        cch = c0 + kk
                        if cch == 0:
                            issue_gather(0)
                            issue_gather(1)
                        if cch + 2 < NCH:
                            issue_gather(cch + 2)
                        xhs, g1 = pending.pop(cch)
                        consumers = []

                        locrep = edg.tile([128, CH], bf16, tag="locrep",
                                         bufs=4)
                        nc.sync.dma_start(out=locrep[:],
                                          in_=bcast(LOCR[cch:cch + 1, :], 128))
                        sel = keep.tile([128, G, 128], bf16, tag="sel")
                        sel2 = keep.tile([128, G, 128], bf16, tag="sel2")
                        rel = keep.tile([128, G, XW], bf16, tag="rel")
                        rdh = keep.tile([128, G, K], bf16, tag="rdh")
                        hsT = keep.tile([F, CH], bf16, tag="hsT")
                        rdT = keep.tile([128, CH], bf16, tag="rdT")
                        fd = keep.tile([128, G, K], bf16, tag="fd")

                        # sel[e, l] = (l == loc_e)   (Pool, per group)
                        for g in range(G):
                            nc.gpsimd.tensor_scalar(
                                sel[:, g, :], iotarow[:],
                                locp_sb[:, cch * G + g:cch * G + g + 1],
                                None, op0=OP.is_equal)
                        # sel2[l, (g,e)] = (l == loc_e)   (DVE, one op)
                        nc.vector.tensor_scalar(
                            sel2[:], locrep[:].rearrange("p (g e) -> p g e",
                                                         g=G),
                            iotacol[:, 0:1], None, op0=OP.is_equal)

                        # xpt[(g,e), :] = XP[loc_e]  (PE expand; bank-
                        # aligned 512-padded groups, two groups per substep)
                        for ss in range(G // 2):
                            xpt = psx.tile([128, 2, 512], f32, space="PSUM",
                                           tag="xpt", bufs=2)
                            for g2 in range(2):
                                g = ss * 2 + g2
                                nc.tensor.matmul(xpt[:, g2, 0:XW],
                                                 sel2[:, g, :],
                                                 xhtb[:, 0:XW],
                                                 start=True, stop=True)
                            i_rel = nc.vector.tensor_sub(
                                rel[:, ss * 2:ss * 2 + 2, :],
                                xhs[:, ss * 2:ss * 2 + 2, 0:XW],
                                xpt[:, :, 0:XW])
                            dep(i_rel, g1, "raw-xhs")
                            consumers.append(i_rel)
                        # rd = sum_c rel^2
                        sq = eds.tile([128, G, XW], bf16, tag="sq",
                                       bufs=3)
                        nc.scalar.activation(sq[:], rel[:], AF.Square)
                        nc.vector.tensor_add(rdh[:], sq[:, :, 0:K],
                                             sq[:, :, K:2 * K])
                        nc.vector.tensor_add(rdh[:], rdh[:], sq[:, :, 2 * K:])
                        # hsT
                        hsp = psht.tile([F, CH], bf16, space="PSUM", tag="hsp")
                        for g in range(G):
                            i_t = nc.tensor.transpose(
                                hsp[:, g * 128:(g + 1) * 128],
                                xhs[:, g, XW:REC], identb[:])
                            dep(i_t, g1, "raw-xhs-h")
                            consumers.append(i_t)
                        nc.scalar.activation(hsT[:], hsp[:], AF.Copy)
                        # rdT
                        rdp = psht.tile([128, CH], bf16, space="PSUM",
                                        tag="tp")
                        for g in range(G):
                            nc.tensor.transpose(rdp[:, g * 128:(g + 1) * 128],
                                                rdh[:, g, :], identb[:])
                        nc.scalar.activation(rdT[:], rdp[:], AF.Copy)
                        # fach = 1 / (1 + sqrt(rd + 1e-8))
                        sxh = eds.tile([128, G, K], bf16, tag="sxh")
                        i_sx = nc.scalar.activation(sxh[:], rdh[:], AF.Sqrt,
                                                    bias=eps8[:])
                        if prev_last_a3[0] is not None:
                            dep(i_sx, prev_last_a3[0], "act-table-phase")
                        last_sx[0] = i_sx
                        fdt = eds.tile([128, G, K], bf16, tag="fdt")
                        nc.vector.tensor_scalar_add(fdt[:], sxh[:], 1.0)
                        nc.vector.reciprocal(fd[:], fdt[:])
                        gath_consumers[cch] = consumers
                        sdata.append((rel, hsT, rdT, fd, sel, sel2))

                    # ---------------- phase M (silu table)
                    for kk in range(nchunks):
                        cch = c0 + kk
                        rel, hsT, rdT, fd, sel, sel2 = sdata[kk]
                        ef = edg.tile([128, CH], bf16, tag="ef", bufs=4)
                        nc.sync.dma_start(out=ef[:],
                                          in_=EF[:, cch * CH:(cch + 1) * CH])

                        z1 = psz.tile([128, CH], f32, space="PSUM", tag="z")
                        nc.tensor.matmul(z1[:F, :], whs[:], hsT[:],
                                         start=True, stop=False)
                        nc.tensor.matmul(z1[:F, :], weate[:], ef[:],
                                         start=False, stop=False)
                        nc.tensor.matmul(z1[:F, :], wrd[:], rdT[:],
                                         start=False, stop=False)
                        for g in range(G):
                            nc.tensor.matmul(z1[:F, g * 128:(g + 1) * 128],
                                             hwb[:], sel2[:, g, :],
                                             start=False, stop=(g == G - 1),
                                             skip_group_check=True)
                        a1 = eds.tile([F, CH], bf16, tag="a1", bufs=3)
                        i_a1 = nc.scalar.activation(a1[:], z1[:F, :], AF.Silu,
                                                    bias=bm1c[:])
                        if last_sx[0] is not None:
                            dep(i_a1, last_sx[0], "act-table-phase")
                        z3 = psz.tile([128, CH], f32, space="PSUM", tag="z")
                        nc.tensor.matmul(z3[:F, :], w23[:], a1[:],
                                         start=True, stop=True)
                        a3 = eds.tile([F, CH], bf16, tag="a3", bufs=3)
                        i_a3 = nc.scalar.activation(a3[:], z3[:F, :], AF.Silu,
                                                    bias=b3c[:])
                        if kk == nchunks - 1:
                            prev_last_a3[0] = i_a3
                        z4 = psz.tile([128, CH], f32, space="PSUM", tag="z")
                        nc.tensor.matmul(z4[:], wx2[:], a3[:],
                                         start=True, stop=True)
                        wt = eds.tile([128, CH], bf16, tag="wt",
                                      bufs=3)
                        nc.vector.tensor_scalar(wt[:], z4[:], bx2c[:, 0:1],
                                                CLAMP, op0=OP.add, op1=OP.min)

                        pwp = psht.tile([128, CH], bf16, space="PSUM",
                                        tag="tp")
                        for g in range(G):
                            nc.tensor.transpose(pwp[:, g * 128:(g + 1) * 128],
                                                wt[:, g * 128:(g + 1) * 128],
                                                identb[:])
                        # fwh = max(pw, -CLAMP) * 1/(1 + sqrt(rd+eps))
                        fwh = eds.tile([128, G, K], bf16, tag="fwh",
                                        bufs=3)
                        nc.vector.scalar_tensor_tensor(
                            fwh[:], pwp[:].rearrange("p (g k) -> p g k", g=G),
                            -CLAMP, fd[:], op0=OP.max, op1=OP.mult)
                        conth = eds.tile([128, G, XW], bf16, tag="conth",
                                          bufs=3)
                        for cc in range(3):
                            nc.vector.tensor_mul(
                                conth[:, :, cc * K:(cc + 1) * K],
                                rel[:, :, cc * K:(cc + 1) * K], fwh[:])
                        for g in range(G):
                            nc.tensor.matmul(upsum[:], sel[:, g, :],
                                             conth[:, g, :],
                                             start=(kk == 0 and g == 0),
                                             stop=(kk == nchunks - 1
                                                   and g == G - 1))

                    # ---------------- block output
                    oj = eds.tile([128, XW], f32, tag="oj")
                    nc.vector.tensor_add(oj[:], upsum[:], xhtb[:, 0:XW])
                    nc.sync.dma_start(out=OUT[b * 128:(b + 1) * 128, :],
                                      in_=oj[:])
                    chunk_base += nchunks

    nc.compile()
    return nc


# ---------------------------------------------------------------- emulation

def emulate_core(cfg, m, sched):
    """bf16-faithful numpy emulation of one core's program."""
    NP, NRP, NB, NCHK = cfg.NP, cfg.NRP, cfg.NB, cfg.NCHK
    NGP = cfg.NG
    nchb = list(sched)
    f32 = np.float32
    Xb = np.asarray(m["X"], f32)       # bf16 values
    Hb = np.asarray(m["H"], f32)
    ohp = np.asarray(m["ohp"], f32)
    oh = ohp.reshape(128, NCHK, NGP).transpose(1, 0, 2).reshape(NP, NGP)
    invc = m["invc"].reshape(NGP)

    ps_m = oh.T @ Xb
    Mneg = bfr(-(ps_m * invc[:, None]))
    xc_all = Xb + oh @ Mneg
    sq = bfr(xc_all ** 2)
    nsq = bfr(bfr(sq[:, :K] + sq[:, K:2 * K]) + sq[:, 2 * K:])
    nrm = bfr(np.sqrt(nsq))
    mnv = (oh.T @ nrm) * invc[:, None] + EPS_E3
    sfb = bfr((1.0 / mnv) * m["e3k"].reshape(1, K))
    sexp_all = oh @ sfb
    XP = bfr(xc_all * np.tile(sexp_all, 3))
    mu = Hb.mean(1, keepdims=True)
    var = ((Hb - mu) ** 2).mean(1, keepdims=True)
    HL = bfr((Hb - mu) / np.sqrt(var + EPS_LN))

    whs = np.asarray(m["whs"], f32)
    wht = np.asarray(m["wht"], f32)
    weate = np.asarray(m["weate"], f32)
    wrd = np.asarray(m["wrd"], f32)
    w23 = np.asarray(m["w23"], f32)
    wx2 = np.asarray(m["wx2"], f32)
    bm1 = m["bm1"].reshape(1, F)
    b3 = m["b3"].reshape(1, F)
    bx2 = m["bx2"].reshape(1, K)

    sidx = m["sidx"][:16].T.reshape(-1).astype(np.int64)
    loc = np.asarray(m["locp"], f32).T.reshape(-1).astype(np.int64)
    ef_all = np.asarray(m["EF"], f32)

    out = np.zeros((NRP, XW), f32)
    silu = lambda z: z / (1.0 + np.exp(-z))
    cch = 0
    for b in range(NB):
        upsum = np.zeros((128, XW), f32)
        XPb = XP[b * 128:(b + 1) * 128]
        HLb = HL[b * 128:(b + 1) * 128]
        hwb = bfr(HLb @ wht)
        for kk in range(nchb[b]):
            sl = slice(cch * CH, (cch + 1) * CH)
            xs = XP[sidx[sl]]
            hs = HL[sidx[sl]]
            lo = loc[sl]
            rel = bfr(xs - XPb[lo])
            sqe = bfr(rel * rel)
            rd = bfr(bfr(sqe[:, :K] + sqe[:, K:2 * K]) + sqe[:, 2 * K:])
            sxh = bfr(np.sqrt(rd + 1e-8))
            fd = bfr(1.0 / bfr(1.0 + sxh))
            ef = ef_all[:, sl].T
            z1 = hs @ whs + hwb[lo] + ef @ weate + rd @ wrd
            a1 = bfr(silu(z1 + bm1))
            z3 = a1 @ w23
            a3 = bfr(silu(z3 + b3))
            wmin = bfr(np.minimum(a3 @ wx2 + bx2, CLAMP))
            fwh = bfr(np.maximum(wmin, -CLAMP) * fd)
            conth = bfr(rel * np.tile(fwh, 3))
            np.add.at(upsum, lo, conth)
            cch += 1
        out[b * 128:(b + 1) * 128] = upsum + XPb
    return out


# ---------------------------------------------------------------- entry point

_PROGRAM_CACHE = {}


def kernel(**inputs):
    """Full-input entry: shards across 8 NeuronCores internally."""
    import sys
    for p in ("/opt/trn_rl_repo", "/root/.axon_site/_ro/trn_rl_repo"):
        if p not in sys.path:
            sys.path.append(p)
    from concourse import bass_utils

    cfg = CFG_FULL
    batch = np.asarray(inputs["batch"]).astype(np.int64)
    X = np.asarray(inputs["X"], np.float32)
    H = np.asarray(inputs["H"], np.float32)
    ei = np.asarray(inputs["edge_index"]).astype(np.int64)
    ea = np.asarray(inputs["edge_attr"], np.float32)
    te = np.asarray(inputs["te"], np.float32)

    shards, sched = build_shards(cfg, ei[0], ei[1], ea, te)
    params = make_params(cfg, *[np.asarray(inputs[k], np.float32) for k in
                         ["Wm1", "bm1", "Wm2", "bm2", "Wx1", "bx1", "Wx2",
                          "bx2", "ln_gamma", "ln_beta", "e3_weight"]])
    in_maps = [prep_core_inputs(cfg, c, shards[c], batch, X, H, params)
               for c in range(cfg.CORES)]

    key = (cfg.N, sched)
    if key not in _PROGRAM_CACHE:
        _PROGRAM_CACHE[key] = build_program(cfg, sched, cfg.CORES)
    nc = _PROGRAM_CACHE[key]

    res = bass_utils.run_bass_kernel_spmd(
        nc, in_maps, core_ids=list(range(cfg.CORES)))
    out = np.zeros((cfg.N, XW), np.float32)
    for c in range(cfg.CORES):
        out[c * cfg.NR:(c + 1) * cfg.NR] = \
            res.results[c]["OUT"][shards[c]["perm"]]
    return out.reshape(cfg.N, 3, K)


# revision 58
# speedup vs baseline: 1.0334x; 1.0005x over previous
"""EquivariantBlock Trainium kernel v2: bf16 + block-aligned chunking.

Layout / sharding:
  - 8 cores, data-parallel by target-node range (2500 nodes each, NRP=2560
    padded). Node tables replicated but ROLLED per core so the core's own
    range sits at rows [0, 2560).
  - Device preprocess (per core, replicated over all NP nodes): per-graph
    centering + E3Norm of X -> XP (bf16), LayerNorm of H -> HLN (bf16,
    gamma/beta folded into MLP weights on host), stored as one record table
    XH = [XP | HLN] ([NP, 448] bf16) in DRAM.
  - Edges sorted by target, grouped into chunks of CH=512 edges where each
    chunk's targets lie inside ONE aligned 128-node block. Per-block chunk
    counts are maxed across cores so all 8 cores share one program.
  - Per chunk: gather src records (bf16, 896B each); target side comes from
    a sequential per-block load + one-hot expansion matmuls (no tgt gather).
    rel is accumulated on the PE (identity matmul + negated-block expand).
    MLP runs feature-major at bf16 (1 cyc/row). Per-128-edge-group one-hot
    segment-sum matmuls accumulate the update in PSUM across all chunks of
    the block; OUT = upsum + XP_block written directly (no UPD table).
  - Act engine table discipline: preproc uses only sqrt-table funcs; the
    edge loop runs a sqrt phase (S) then a silu phase (M) per block, so the
    compiler inserts only 2 act-table loads per block.
"""
import numpy as np
import ml_dtypes

BF16 = ml_dtypes.bfloat16

F = 64
K = 128
XW = 3 * K          # 384
REC = XW + F        # 448
RECG = 512          # gather record (padded: 512*2B is a multiple of 256)
CH = 512            # edges per chunk
G = CH // 128       # groups per chunk
EPS_E3 = 1e-5
EPS_LN = 1e-5
CLAMP = 10.0


def bf(a):
    return np.ascontiguousarray(np.asarray(a, np.float32).astype(BF16))


def bfr(a):
    """Round to bf16, return fp32 (for emulation)."""
    return np.asarray(a, np.float32).astype(BF16).astype(np.float32)


class Cfg:
    def __init__(self, n_nodes, n_graphs, cores):
        self.N = n_nodes
        self.NG = n_graphs
        self.CORES = cores
        self.NR = n_nodes // cores        # nodes per core
        self.NRP = -(-self.NR // 128) * 128
        self.NB = self.NRP // 128         # target blocks per core
        self.NCHK = -(-n_nodes // 128)    # node chunks for preprocess
        self.NP = self.NCHK * 128


CFG_FULL = Cfg(20000, 64, 8)


# ---------------------------------------------------------------- host prep

def build_shards(cfg, src, tgt, edge_attr, te):
    """Partition edges by target block; schedule shared across cores."""
    N, NR, NB, CORES = cfg.N, cfg.NR, cfg.NB, cfg.CORES
    percore = []
    for c in range(CORES):
        em = np.where(np.minimum(tgt // NR, CORES - 1) == c)[0]
        tl0 = (tgt[em] - c * NR).astype(np.int64)
        nr_here = N - c * NR if c == CORES - 1 else NR
        deg = np.bincount(tl0, minlength=NR)
        # greedy balance: assign nodes (desc degree) to blocks, cap 128
        # nodes and minimal edge total per block -> all blocks ~N_edges/NB
        order_nodes = np.argsort(-deg, kind="stable")
        blk_of = np.zeros(NR, np.int64)
        slot_of = np.zeros(NR, np.int64)
        btot = np.zeros(NB, np.int64)
        bcnt = np.zeros(NB, np.int64)
        for v in order_nodes:
            cand = np.where(bcnt < 128)[0]
            bsel = cand[np.argmin(btot[cand])]
            blk_of[v] = bsel
            slot_of[v] = bcnt[bsel]
            btot[bsel] += deg[v]
            bcnt[bsel] += 1
        perm = blk_of * 128 + slot_of          # old local id -> new local id
        iperm = np.zeros(cfg.NRP, np.int64)
        iperm[perm] = np.arange(NR)            # new local id -> old local id
        tl = perm[tl0]
        order = np.argsort(tl, kind="stable")
        eidx = em[order]
        tl = tl[order]
        sg = src[eidx]
        blk = tl // 128
        cnt = np.bincount(blk, minlength=NB)
        percore.append(dict(eidx=eidx, tl=tl, sg=sg, cnt=cnt,
                            perm=perm, iperm=iperm))

    # shared schedule: chunks per block = max over cores
    nchb = [max(1, int(-(-max(pc["cnt"][b] for pc in percore) // CH)))
            for b in range(NB)]
    sched = tuple(nchb)
    NCH = sum(nchb)
    LE = NCH * CH

    shards = []
    for c in range(CORES):
        pc = percore[c]
        eidx, tl, sg, cnt = pc["eidx"], pc["tl"], pc["sg"], pc["cnt"]
        perm = pc["perm"]
        starts = np.concatenate([[0], np.cumsum(cnt)])
        sidx = np.zeros(LE, np.int64)
        loc = np.zeros(LE, np.int64)
        epos = np.full(LE, -1, np.int64)
        pos = 0
        for b in range(NB):
            e0, e1 = starts[b], starts[b + 1]
            ne = e1 - e0
            cap = nchb[b] * CH
            assert ne <= cap, f"block {b} core {c}: {ne} > {cap}"
            sl = slice(pos, pos + ne)
            sr = (sg[e0:e1] - c * NR) % N      # rolled row of source
            own = sr < NR
            sr = np.where(own, 0, sr + (cfg.NRP - NR))
            sr[own] = perm[((sg[e0:e1] - c * NR) % N)[own]]
            sidx[sl] = sr
            loc[sl] = tl[e0:e1] - b * 128
            epos[sl] = eidx[e0:e1]
            dl = slice(pos + ne, pos + cap)
            sidx[dl] = b * 128
            loc[dl] = 0
            pos += cap
        assert pos == LE

        ef = np.zeros((128, LE), np.float32)
        valid = epos >= 0
        ef[:F, valid] = edge_attr[epos[valid]].T
        ef[F:, valid] = te[epos[valid]].T

        def wrap16(v):
            return np.ascontiguousarray(
                np.tile(v.astype(np.int16).reshape(-1, 16).T, (8, 1)))

        shards.append(dict(
            sidx16=wrap16(sidx), perm=perm, iperm=pc["iperm"],
            locp=np.ascontiguousarray(
                loc.reshape(-1, 128).T.astype(np.float32)),
            locr=bf(loc.reshape(NCH, CH)),              # [NCH, 512]
            ef=bf(ef),
            sidx=sidx, loc=loc, epos=epos,
        ))
    return shards, sched


def make_params(cfg, Wm1, bm1, Wm2, bm2, Wx1, bx1, Wx2, bx2, ln_gamma, ln_beta,
                e3_weight):
    f = np.float32
    Wm1 = np.asarray(Wm1, f)
    g = np.asarray(ln_gamma, f).reshape(F)
    bt = np.asarray(ln_beta, f).reshape(F)
    W_ht = Wm1[0:F] * g[:, None]          # fold LN gamma into H weights
    W_hs = Wm1[F:2 * F] * g[:, None]
    b1 = (np.asarray(bm1, f).reshape(F)
          + bt @ Wm1[0:F] + bt @ Wm1[F:2 * F])   # fold LN beta
    b3 = (np.asarray(bx1, f).reshape(F)
          + np.asarray(bm2, f).reshape(F) @ np.asarray(Wx1, f))  # fold bm2
    return dict(
        whs=bf(W_hs),
        wht=bf(W_ht),
        weate=bf(np.concatenate([Wm1[2 * F:3 * F], Wm1[2 * F + K + F:]], 0)),
        wrd=bf(Wm1[3 * F:3 * F + K]),                       # [128, 64]
        w23=bf(np.asarray(Wm2, f) @ np.asarray(Wx1, f)),
        wx2=bf(Wx2),
        bm1=np.ascontiguousarray(b1.reshape(F, 1), f),
        b3=np.ascontiguousarray(b3.reshape(F, 1), f),
        bx2=np.ascontiguousarray(np.asarray(bx2, f).reshape(K, 1), f),
        e3k=np.ascontiguousarray(np.asarray(e3_weight, f).reshape(1, K), f),
    )


def make_onehots(cfg, bf_ids):
    NGP = cfg.NG
    oh = (bf_ids[:, None] == np.arange(NGP)[None, :]).astype(np.float32)
    ohp = np.ascontiguousarray(
        oh.reshape(cfg.NCHK, 128, NGP).transpose(1, 0, 2).reshape(128, -1))
    oht = np.ascontiguousarray(oh.T)
    return bf(ohp), bf(oht), NGP


def prep_core_inputs(cfg, c, shard, batch, X, H, params):
    N, NR, NP, NRP = cfg.N, cfg.NR, cfg.NP, cfg.NRP
    roll = lambda a: np.roll(a, -c * NR, axis=0)
    Xp = np.zeros((NP, XW), np.float32)
    Xp[:N] = roll(np.asarray(X, np.float32).reshape(N, XW))
    Hp = np.zeros((NP, F), np.float32)
    Hp[:N] = roll(np.asarray(H, np.float32))
    bf_ids = np.full(NP, -1.0, np.float32)
    bf_ids[:N] = roll(np.asarray(batch)).astype(np.float32)
    # permute own range into degree-balanced block order; shift the rest
    # of the rolled table up by NRP-NR rows so nothing is clobbered
    perm = shard["perm"]
    off = NRP - NR
    for arr, fill in ((Xp, 0.0), (Hp, 0.0), (bf_ids, -1.0)):
        ownnew = np.full((NRP,) + arr.shape[1:], fill, arr.dtype)
        ownnew[perm] = arr[:NR].copy()
        rest = arr[NR:NP - off].copy()
        arr[:NRP] = ownnew
        arr[NRP:NP] = rest
    ohp, oht, NGP = make_onehots(cfg, bf_ids)
    cnts = np.zeros(NGP, np.float64)
    gg, n = np.unique(np.asarray(batch), return_counts=True)
    cnts[gg.astype(np.int64)] = n
    invc_h = (1.0 / np.maximum(cnts, 1.0)).astype(np.float32).reshape(NGP, 1)
    m = dict(X=bf(Xp), H=Hp, ohp=ohp, oht=oht, invc=invc_h,
             sidx=shard["sidx16"], locp=shard["locp"], locr=shard["locr"],
             EF=shard["ef"])
    m.update(params)
    return m


# ---------------------------------------------------------------- device program

def build_program(cfg, sched, num_devices):
    import concourse.bacc as bacc
    import concourse.bass as bass
    import concourse.tile as tile
    from concourse import mybir
    from concourse.masks import make_identity
    from concourse.tile import add_dep_helper

    def dep(a, b, why):
        add_dep_helper(a.ins, b.ins, sync=True, reason=why)

    f32 = mybir.dt.float32
    bf16 = mybir.dt.bfloat16
    i16 = mybir.dt.int16
    AF = mybir.ActivationFunctionType
    OP = mybir.AluOpType

    N, NP, NCHK, NRP, NB = cfg.N, cfg.NP, cfg.NCHK, cfg.NRP, cfg.NB
    NGP = cfg.NG
    nchb = list(sched)
    NCH = sum(nchb)
    LE = NCH * CH
    NCB = max(nchb)

    nc = bacc.Bacc("TRN2", target_bir_lowering=False, debug=False,
                   num_devices=num_devices)

    def din(name, shape, dt=bf16):
        return nc.dram_tensor(name, shape, dt, kind="ExternalInput").ap()

    X = din("X", [NP, XW])
    H = din("H", [NP, F], f32)
    OHP = din("ohp", [128, NCHK * NGP])
    OHT = din("oht", [NGP, NP])
    INVC = din("invc", [NGP, 1], f32)
    SIDX = din("sidx", [128, LE // 16], i16)
    LOCP = din("locp", [128, LE // 128], f32)
    LOCR = din("locr", [NCH, CH])
    EF = din("EF", [128, LE])
    WHS = din("whs", [F, F])
    WHT = din("wht", [F, F])
    WEATE = din("weate", [2 * F, F])
    WRD = din("wrd", [K, F])
    W23 = din("w23", [F, F])
    WX2 = din("wx2", [F, K])
    BM1 = din("bm1", [F, 1], f32)
    B3 = din("b3", [F, 1], f32)
    BX2 = din("bx2", [K, 1], f32)
    E3K = din("e3k", [1, K], f32)

    OUT = nc.dram_tensor("OUT", [NRP, XW], f32, kind="ExternalOutput").ap()

    def bcast(dram_ap, parts):
        return bass.AP(tensor=dram_ap.tensor, offset=dram_ap.offset,
                       ap=[[0, parts]] + [list(p) for p in dram_ap.ap[1:]])

    def rep_mid(ap, n):
        """[P, W] -> [P, n, W] with a stride-0 middle dim."""
        aps = [list(p) for p in ap.ap]
        return bass.AP(tensor=ap.tensor, offset=ap.offset,
                       ap=aps[:-1] + [[0, n]] + aps[-1:])

    import contextlib
    with tile.TileContext(nc) as tc, \
         nc.allow_low_precision(reason="bf16 kernel, 2e-2 tolerance"), \
         contextlib.ExitStack() as ctx:
        if True:
            const = ctx.enter_context(tc.tile_pool(name="const", bufs=1))
            dramp = ctx.enter_context(tc.tile_pool(name="dram", bufs=1, space="DRAM"))

            XH = dramp.tile([NP, RECG], bf16)

            # ---- constants
            identb = const.tile([128, 128], bf16)
            make_identity(nc, identb[:])
            iotacol_i = const.tile([128, 1], mybir.dt.int32)
            nc.gpsimd.iota(iotacol_i[:], pattern=[[1, 1]], base=0,
                           channel_multiplier=1)
            iotacol = const.tile([128, 1], f32)
            nc.vector.tensor_copy(iotacol[:], iotacol_i[:])
            iotarow_i = const.tile([128, 128], mybir.dt.int32)
            nc.gpsimd.iota(iotarow_i[:], pattern=[[1, 128]], base=0,
                           channel_multiplier=0)
            iotarow = const.tile([128, 128], bf16)
            nc.vector.tensor_copy(iotarow[:], iotarow_i[:])

            whs = const.tile([F, F], bf16)
            nc.sync.dma_start(out=whs[:], in_=WHS)
            wht = const.tile([F, F], bf16)
            nc.sync.dma_start(out=wht[:], in_=WHT)
            weate = const.tile([2 * F, F], bf16)
            nc.sync.dma_start(out=weate[:], in_=WEATE)
            wrd = const.tile([K, F], bf16)
            nc.sync.dma_start(out=wrd[:], in_=WRD)
            w23 = const.tile([F, F], bf16)
            nc.sync.dma_start(out=w23[:], in_=W23)
            wx2 = const.tile([F, K], bf16)
            nc.sync.dma_start(out=wx2[:], in_=WX2)
            bm1c = const.tile([F, 1], f32)
            nc.sync.dma_start(out=bm1c[:], in_=BM1)
            b3c = const.tile([F, 1], f32)
            nc.sync.dma_start(out=b3c[:], in_=B3)
            bx2c = const.tile([K, 1], f32)
            nc.sync.dma_start(out=bx2c[:], in_=BX2)
            e3b = const.tile([NGP, K], f32)
            nc.sync.dma_start(out=e3b[:], in_=bcast(E3K, NGP))
            eps8 = const.tile([128, 1], f32)
            nc.vector.memset(eps8[:], 1e-8)
            epsln = const.tile([128, 1], f32)
            nc.vector.memset(epsln[:], EPS_LN)

            Mneg = const.tile([NGP, XW], bf16)
            sfb = const.tile([NGP, K], bf16)
            invc = const.tile([NGP, 1], f32)
            zpad8 = const.tile([128, 8, RECG - REC], bf16)
            nc.vector.memset(zpad8[:], 0.0)

            # ---- preprocessing (sqrt act-table only)
            with tc.tile_pool(name="xall", bufs=1) as xap, \
                 tc.tile_pool(name="preoh", bufs=1) as poh, \
                 tc.tile_pool(name="pre", bufs=4) as pre, \
                 tc.tile_pool(name="prep", bufs=2, space="PSUM") as pps, \
                 tc.tile_pool(name="preacc", bufs=1, space="PSUM") as pacc:

                ohp_sb = poh.tile([128, NCHK * NGP], bf16)
                nc.sync.dma_start(out=ohp_sb[:], in_=OHP)
                nc.sync.dma_start(out=invc[:], in_=INVC)

                groups = [(cb, min(4, NCHK - cb)) for cb in range(0, NCHK, 4)]

                xall = xap.tile([128, NCHK, XW], bf16)
                for cb, nb in groups:
                    nc.sync.dma_start(
                        out=xall[:, cb:cb + nb, :],
                        in_=X[cb * 128:(cb + nb) * 128, :].rearrange(
                            "(c p) w -> p c w", p=128))

                # P1: per-graph sums of X -> M_mean (negated bf16)
                ps_m = pacc.tile([NGP, XW], f32, space="PSUM")
                for ci in range(NCHK):
                    nc.tensor.matmul(ps_m[:],
                                     ohp_sb[:, ci * NGP:(ci + 1) * NGP],
                                     xall[:, ci, :], start=(ci == 0),
                                     stop=(ci == NCHK - 1))
                mmf = pre.tile([NGP, XW], f32, tag="mmf")
                nc.vector.tensor_scalar_mul(mmf[:], ps_m[:], invc[:, 0:1])
                nc.scalar.activation(Mneg[:], mmf[:], AF.Copy, scale=-1.0)

                # P2: per-graph mean vector-norm (ops batched over 4 chunks)
                ps_n = pacc.tile([NGP, K], f32, space="PSUM")
                for cb, nb in groups:
                    oht4 = pre.tile([NGP, 4, 128], bf16, tag="oht4")
                    nc.sync.dma_start(
                        out=oht4[:, :nb, :],
                        in_=OHT[:, cb * 128:(cb + nb) * 128].rearrange(
                            "g (c p) -> g c p", p=128))
                    xc4 = pps.tile([128, 4, 512], f32, space="PSUM", tag="xc",
                                   bufs=1)
                    for j in range(nb):
                        ci = cb + j
                        nc.tensor.matmul(xc4[:, j, 0:XW], oht4[:, j, :],
                                         Mneg[:], start=True, stop=False)
                        nc.tensor.matmul(xc4[:, j, 0:XW], identb[:],
                                         xall[:, ci, :],
                                         start=False, stop=True)
                    sq4 = pre.tile([128, 4, XW], bf16, tag="sq")
                    nc.scalar.activation(sq4[:, :nb, :], xc4[:, :nb, 0:XW],
                                         AF.Square)
                    nsq4 = pre.tile([128, 4, K], bf16, tag="nsq")
                    nc.vector.tensor_add(nsq4[:, :nb, :], sq4[:, :nb, 0:K],
                                         sq4[:, :nb, K:2 * K])
                    nc.vector.tensor_add(nsq4[:, :nb, :], nsq4[:, :nb, :],
                                         sq4[:, :nb, 2 * K:])
                    nrm4 = pre.tile([128, 4, K], bf16, tag="nrm")
                    nc.scalar.activation(nrm4[:, :nb, :], nsq4[:, :nb, :],
                                         AF.Sqrt)
                    for j in range(nb):
                        ci = cb + j
                        nc.tensor.matmul(ps_n[:],
                                         ohp_sb[:, ci * NGP:(ci + 1) * NGP],
                                         nrm4[:, j, :], start=(ci == 0),
                                         stop=(ci == NCHK - 1))
                mn = pre.tile([NGP, K], f32, tag="mn")
                nc.vector.tensor_scalar(mn[:], ps_n[:], invc[:, 0:1], EPS_E3,
                                        op0=OP.mult, op1=OP.add)
                rmn = pre.tile([NGP, K], f32, tag="rmn")
                nc.vector.reciprocal(rmn[:], mn[:])
                sff = pre.tile([NGP, K], f32, tag="sff")
                nc.vector.tensor_mul(sff[:], rmn[:], e3b[:])
                nc.scalar.activation(sfb[:], sff[:], AF.Copy)

                # P3: XP = (X - M[g]) * sfac[g]  -> XH[:, :XW]
                for cb, nb in groups:
                    oht4 = pre.tile([NGP, 4, 128], bf16, tag="oht4")
                    nc.sync.dma_start(
                        out=oht4[:, :nb, :],
                        in_=OHT[:, cb * 128:(cb + nb) * 128].rearrange(
                            "g (c p) -> g c p", p=128))
                    xp4 = pre.tile([128, 4, XW], bf16, tag="xp4")
                    xc4 = pps.tile([128, 4, 512], f32, space="PSUM", tag="xc",
                                   bufs=1)
                    sexp4 = pps.tile([128, 4, K], f32, space="PSUM",
                                     tag="sexp", bufs=1)
                    for j in range(nb):
                        ci = cb + j
                        nc.tensor.matmul(xc4[:, j, 0:XW], oht4[:, j, :],
                                         Mneg[:], start=True, stop=False)
                        nc.tensor.matmul(xc4[:, j, 0:XW], identb[:],
                                         xall[:, ci, :],
                                         start=False, stop=True)
                        nc.tensor.matmul(sexp4[:, j, :], oht4[:, j, :],
                                         sfb[:], start=True, stop=True)
                    sxb4 = pre.tile([128, 4, K], bf16, tag="sxb")
                    nc.vector.tensor_copy(sxb4[:, :nb, :], sexp4[:, :nb, :])
                    for j in range(nb):
                        nc.vector.scalar_tensor_tensor(
                            xp4[:, j, :].rearrange("p (d k) -> p d k", d=3),
                            xc4[:, j, 0:XW].rearrange("p (d k) -> p d k", d=3),
                            0.0, rep_mid(sxb4[:, j, :], 3),
                            op0=OP.bypass, op1=OP.mult)
                    nc.sync.dma_start(
                        out=XH[cb * 128:(cb + nb) * 128, 0:XW].rearrange(
                            "(c p) w -> p c w", p=128),
                        in_=xp4[:, :nb, :])

                # P4: HLN (no gamma/beta: folded into weights) -> XH[:, XW:]
                hgroups = [(cb, min(8, NCHK - cb)) for cb in range(0, NCHK, 8)]
                for cb, nb in hgroups:
                    h8 = pre.tile([128, 8, F], f32, tag="h8")
                    nc.sync.dma_start(
                        out=h8[:, :nb, :],
                        in_=H[cb * 128:(cb + nb) * 128, :].rearrange(
                            "(c p) w -> p c w", p=128))
                    hg8 = pre.tile([128, 8, F], bf16, tag="hg8")
                    for j in range(nb):
                        ht = h8[:, j, :]
                        st = pre.tile([128, 6], f32, tag="st")
                        nc.vector.bn_stats(out=st[:], in_=ht)
                        mv = pre.tile([128, 2], f32, tag="mv")
                        nc.vector.bn_aggr(out=mv[:], in_=st[:])
                        sd = pre.tile([128, 1], f32, tag="sd")
                        nc.scalar.activation(sd[:], mv[:, 1:2], AF.Sqrt,
                                             bias=epsln[:])
                        rs = pre.tile([128, 1], f32, tag="rs")
                        nc.vector.reciprocal(rs[:], sd[:])
                        nc.vector.tensor_scalar(hg8[:, j, :], ht, mv[:, 0:1],
                                                rs[:, 0:1],
                                                op0=OP.subtract, op1=OP.mult)
                    nc.sync.dma_start(
                        out=XH[cb * 128:(cb + nb) * 128, XW:REC].rearrange(
                            "(c p) w -> p c w", p=128),
                        in_=hg8[:, :nb, :])
                    nc.sync.dma_start(
                        out=XH[cb * 128:(cb + nb) * 128, REC:RECG].rearrange(
                            "(c p) w -> p c w", p=128),
                        in_=zpad8[:, :nb, :])

            # ---- edge loop
            with tc.tile_pool(name="edi", bufs=1) as edi, \
                 tc.tile_pool(name="blk", bufs=2) as blkp, \
                 tc.tile_pool(name="edg", bufs=3) as edg, \
                 tc.tile_pool(name="eds", bufs=2) as eds, \
                 tc.tile_pool(name="keep", bufs=NCB + 5) as keep, \
                 tc.tile_pool(name="psx", bufs=1, space="PSUM") as psx, \
                 tc.tile_pool(name="psht", bufs=1, space="PSUM") as psht, \
                 tc.tile_pool(name="psz", bufs=1, space="PSUM") as psz, \
                 tc.tile_pool(name="psu", bufs=1, space="PSUM") as psu:

                sidx_sb = edi.tile([128, LE // 16], i16)
                nc.sync.dma_start(out=sidx_sb[:], in_=SIDX)
                locp_sb = edi.tile([128, LE // 128], f32)
                nc.sync.dma_start(out=locp_sb[:], in_=LOCP)

                # preprocess XH writes land before gathers (invisible APs)
                tc.strict_bb_all_engine_barrier()

                IC = CH // 16
                gidx_reg = nc.gpsimd.alloc_register("gidx")
                nc.gpsimd.reg_mov(gidx_reg, CH)
                gath_consumers = {}
                pending = {}
                GB = 3  # xhs ring depth

                def issue_gather(cch):
                    war = gath_consumers.pop(cch - GB, None)
                    xhs = edg.tile([128, G, RECG], bf16, tag="xhs",
                                   name=f"xhs{cch}")
                    g1 = nc.gpsimd.dma_gather(
                        out_ap=xhs[:], in_ap=XH[:],
                        idxs_ap=sidx_sb[:, cch * IC:(cch + 1) * IC],
                        num_idxs=CH, num_idxs_reg=gidx_reg, elem_size=RECG,
                        single_packet=False)
                    if war:
                        for ci in war:
                            dep(g1, ci, "war-xhs")
                    pending[cch] = (xhs, g1)

                chunk_base = 0
                prev_last_a3 = [None]
                last_sx = [None]
                for b in range(NB):
                    nchunks = nchb[b]
                    c0 = chunk_base

                    xhtb = blkp.tile([128, RECG], bf16, tag="xhtb")
                    nc.sync.dma_start(out=xhtb[:],
                                      in_=XH[b * 128:(b + 1) * 128, :])

                    # HW = Hblk @ W_ht  (per block; ht-term enters z1 via sel2)
                    hbtp = psht.tile([F, CH], bf16, space="PSUM", tag="hsp")
                    nc.tensor.transpose(hbtp[:, 0:128], xhtb[:, XW:REC],
                                        identb[:])
                    hbt = blkp.tile([F, 128], bf16, tag="hbt")
                    nc.scalar.activation(hbt[:], hbtp[:, 0:128], AF.Copy)
                    hwp = psz.tile([128, CH], f32, space="PSUM", tag="z")
                    nc.tensor.matmul(hwp[:, 0:F], hbt[:], wht[:],
                                     start=True, stop=True)
                    hwb = blkp.tile([128, F], bf16, tag="hwb")
                    nc.scalar.activation(hwb[:], hwp[:, 0:F], AF.Copy)

                    upsum = psu.tile([128, XW], f32, space="PSUM", tag="u",
                                     bufs=1)

                    # ---------------- phase S (sqrt table)
                    sdata = []
                    for kk in range(nchunks):
                        cch = c0 + kk
                        if cch == 0:
                            issue_gather(0)
                            issue_gather(1)
                        if cch + 2 < NCH:
                            issue_gather(cch + 2)
                        xhs, g1 = pending.pop(cch)
                        consumers = []

                        locrep = edg.tile([128, CH], bf16, tag="locrep",
                                         bufs=4)
                        nc.sync.dma_start(out=locrep[:],
                                          in_=bcast(LOCR[cch:cch + 1, :], 128))
                        sel = keep.tile([128, G, 128], bf16, tag="sel")
                        sel2 = keep.tile([128, G, 128], bf16, tag="sel2")
                        rel = keep.tile([128, G, XW], bf16, tag="rel")
                        rdh = keep.tile([128, G, K], bf16, tag="rdh")
                        hsT = keep.tile([F, CH], bf16, tag="hsT")
                        rdT = keep.tile([128, CH], bf16, tag="rdT")
                        fd = keep.tile([128, G, K], bf16, tag="fd")

                        # sel[e, l] = (l == loc_e)   (Pool, per group)
                        for g in range(G):
                            nc.gpsimd.tensor_scalar(
                                sel[:, g, :], iotarow[:],
                                locp_sb[:, cch * G + g:cch * G + g + 1],
                                None, op0=OP.is_equal)
                        # sel2[l, (g,e)] = (l == loc_e)   (DVE, one op)
                        nc.vector.tensor_scalar(
                            sel2[:], locrep[:].rearrange("p (g e) -> p g e",
                                                         g=G),
                            iotacol[:, 0:1], None, op0=OP.is_equal)

                        # xpt[(g,e), :] = XP[loc_e]  (PE expand; bank-
                        # aligned 512-padded groups, two groups per substep)
                        for ss in range(G // 2):
                            xpt = psx.tile([128, 2, 512], f32, space="PSUM",
                                           tag="xpt", bufs=2)
                            for g2 in range(2):
                                g = ss * 2 + g2
                                nc.tensor.matmul(xpt[:, g2, 0:XW],
                                                 sel2[:, g, :],
                                                 xhtb[:, 0:XW],
                                                 start=True, stop=True)
                            i_rel = nc.vector.tensor_sub(
                                rel[:, ss * 2:ss * 2 + 2, :],
                                xhs[:, ss * 2:ss * 2 + 2, 0:XW],
                                xpt[:, :, 0:XW])
                            dep(i_rel, g1, "raw-xhs")
                            consumers.append(i_rel)
                        # rd = sum_c rel^2
                        sq = eds.tile([128, G, XW], bf16, tag="sq",
                                       bufs=3)
                        nc.scalar.activation(sq[:], rel[:], AF.Square)
                        nc.vector.tensor_add(rdh[:], sq[:, :, 0:K],
                                             sq[:, :, K:2 * K])
                        nc.vector.tensor_add(rdh[:], rdh[:], sq[:, :, 2 * K:])
                        # hsT
                        hsp = psht.tile([F, CH], bf16, space="PSUM", tag="hsp")
                        for g in range(G):
                            i_t = nc.tensor.transpose(
                                hsp[:, g * 128:(g + 1) * 128],
                                xhs[:, g, XW:REC], identb[:])
                            dep(i_t, g1, "raw-xhs-h")
                            consumers.append(i_t)
                        nc.scalar.activation(hsT[:], hsp[:], AF.Copy)
                        # rdT
                        rdp = psht.tile([128, CH], bf16, space="PSUM",
                                        tag="tp")
                        for g in range(G):
                            nc.tensor.transpose(rdp[:, g * 128:(g + 1) * 128],
                                                rdh[:, g, :], identb[:])
                        nc.scalar.activation(rdT[:], rdp[:], AF.Copy)
                        # fach = 1 / (1 + sqrt(rd + 1e-8))
                        sxh = eds.tile([128, G, K], bf16, tag="sxh")
                        i_sx = nc.scalar.activation(sxh[:], rdh[:], AF.Sqrt,
                                                    bias=eps8[:])
                        if prev_last_a3[0] is not None:
                            dep(i_sx, prev_last_a3[0], "act-table-phase")
                        last_sx[0] = i_sx
                        fdt = eds.tile([128, G, K], bf16, tag="fdt")
                        nc.vector.tensor_scalar_add(fdt[:], sxh[:], 1.0)
                        nc.vector.reciprocal(fd[:], fdt[:])
                        gath_consumers[cch] = consumers
                        sdata.append((rel, hsT, rdT, fd, sel, sel2))

                    # ---------------- phase M (silu table)
                    for kk in range(nchunks):
                        cch = c0 + kk
                        rel, hsT, rdT, fd, sel, sel2 = sdata[kk]
                        ef = edg.tile([128, CH], bf16, tag="ef", bufs=4)
                        nc.sync.dma_start(out=ef[:],
                                          in_=EF[:, cch * CH:(cch + 1) * CH])

                        z1 = psz.tile([128, CH], f32, space="PSUM", tag="z")
                        nc.tensor.matmul(z1[:F, :], whs[:], hsT[:],
                                         start=True, stop=False)
                        nc.tensor.matmul(z1[:F, :], weate[:], ef[:],
                                         start=False, stop=False)
                        nc.tensor.matmul(z1[:F, :], wrd[:], rdT[:],
                                         start=False, stop=False)
                        for g in range(G):
                            nc.tensor.matmul(z1[:F, g * 128:(g + 1) * 128],
                                             hwb[:], sel2[:, g, :],
                                             start=False, stop=(g == G - 1),
                                             skip_group_check=True)
                        a1 = eds.tile([F, CH], bf16, tag="a1", bufs=3)
                        i_a1 = nc.scalar.activation(a1[:], z1[:F, :], AF.Silu,
                                                    bias=bm1c[:])
                        if last_sx[0] is not None:
                            dep(i_a1, last_sx[0], "act-table-phase")
                        z3 = psz.tile([128, CH], f32, space="PSUM", tag="z")
                        nc.tensor.matmul(z3[:F, :], w23[:], a1[:],
                                         start=True, stop=True)
                        a3 = eds.tile([F, CH], bf16, tag="a3", bufs=3)
                        i_a3 = nc.scalar.activation(a3[:], z3[:F, :], AF.Silu,
                                                    bias=b3c[:])
                        if kk == nchunks - 1:
                            prev_last_a3[0] = i_a3
                        z4 = psz.tile([128, CH], f32, space="PSUM", tag="z")
                        nc.tensor.matmul(z4[:], wx2[:], a3[:],
                                         start=True, stop=True)
                        wt = eds.tile([128, CH], bf16, tag="wt",
                                      bufs=3)
                        nc.vector.tensor_scalar(wt[:], z4[:], bx2c[:, 0:1],
                                                CLAMP, op0=OP.add, op1=OP.min)

                        pwp = psht.tile([128, CH], bf16, space="PSUM",
                                        tag="tp")
                        for g in range(G):
                            nc.tensor.transpose(pwp[:, g * 128:(g + 1) * 128],
                                                wt[:, g * 128:(g + 1) * 128],
                                                identb[:])
                        # fwh = max(pw, -CLAMP) * 1/(1 + sqrt(rd+eps))
                        fwh = eds.tile([128, G, K], bf16, tag="fwh",
                                        bufs=3)
                        nc.vector.scalar_tensor_tensor(
                            fwh[:], pwp[:].rearrange("p (g k) -> p g k", g=G),
                            -CLAMP, fd[:], op0=OP.max, op1=OP.mult)
                        conth = eds.tile([128, G, XW], bf16, tag="conth",
                                          bufs=3)
                        for cc in range(3):
                            nc.vector.tensor_mul(
                                conth[:, :, cc * K:(cc + 1) * K],
                                rel[:, :, cc * K:(cc + 1) * K], fwh[:])
                        for g in range(G):
                            nc.tensor.matmul(upsum[:], sel[:, g, :],
                                             conth[:, g, :],
                                             start=(kk == 0 and g == 0),
                                             stop=(kk == nchunks - 1
                                                   and g == G - 1))

                    # ---------------- block output
                    oj = eds.tile([128, XW], f32, tag="oj", bufs=3)
                    nc.vector.tensor_add(oj[:], upsum[:], xhtb[:, 0:XW])
                    nc.sync.dma_start(out=OUT[b * 128:(b + 1) * 128, :],
                                      in_=oj[:])
                    chunk_base += nchunks

    nc.compile()
    return nc


# ---------------------------------------------------------------- emulation

def emulate_core(cfg, m, sched):
    """bf16-faithful numpy emulation of one core's program."""
    NP, NRP, NB, NCHK = cfg.NP, cfg.NRP, cfg.NB, cfg.NCHK
    NGP = cfg.NG
    nchb = list(sched)
    f32 = np.float32
    Xb = np.asarray(m["X"], f32)       # bf16 values
    Hb = np.asarray(m["H"], f32)
    ohp = np.asarray(m["ohp"], f32)
    oh = ohp.reshape(128, NCHK, NGP).transpose(1, 0, 2).reshape(NP, NGP)
    invc = m["invc"].reshape(NGP)

    ps_m = oh.T @ Xb
    Mneg = bfr(-(ps_m * invc[:, None]))
    xc_all = Xb + oh @ Mneg
    sq = bfr(xc_all ** 2)
    nsq = bfr(bfr(sq[:, :K] + sq[:, K:2 * K]) + sq[:, 2 * K:])
    nrm = bfr(np.sqrt(nsq))
    mnv = (oh.T @ nrm) * invc[:, None] + EPS_E3
    sfb = bfr((1.0 / mnv) * m["e3k"].reshape(1, K))
    sexp_all = oh @ sfb
    XP = bfr(xc_all * np.tile(sexp_all, 3))
    mu = Hb.mean(1, keepdims=True)
    var = ((Hb - mu) ** 2).mean(1, keepdims=True)
    HL = bfr((Hb - mu) / np.sqrt(var + EPS_LN))

    whs = np.asarray(m["whs"], f32)
    wht = np.asarray(m["wht"], f32)
    weate = np.asarray(m["weate"], f32)
    wrd = np.asarray(m["wrd"], f32)
    w23 = np.asarray(m["w23"], f32)
    wx2 = np.asarray(m["wx2"], f32)
    bm1 = m["bm1"].reshape(1, F)
    b3 = m["b3"].reshape(1, F)
    bx2 = m["bx2"].reshape(1, K)

    sidx = m["sidx"][:16].T.reshape(-1).astype(np.int64)
    loc = np.asarray(m["locp"], f32).T.reshape(-1).astype(np.int64)
    ef_all = np.asarray(m["EF"], f32)

    out = np.zeros((NRP, XW), f32)
    silu = lambda z: z / (1.0 + np.exp(-z))
    cch = 0
    for b in range(NB):
        upsum = np.zeros((128, XW), f32)
        XPb = XP[b * 128:(b + 1) * 128]
        HLb = HL[b * 128:(b + 1) * 128]
        hwb = bfr(HLb @ wht)
        for kk in range(nchb[b]):
            sl = slice(cch * CH, (cch + 1) * CH)
            xs = XP[sidx[sl]]
            hs = HL[sidx[sl]]
            lo = loc[sl]
            rel = bfr(xs - XPb[lo])
            sqe = bfr(rel * rel)
            rd = bfr(bfr(sqe[:, :K] + sqe[:, K:2 * K]) + sqe[:, 2 * K:])
            sxh = bfr(np.sqrt(rd + 1e-8))
            fd = bfr(1.0 / bfr(1.0 + sxh))
            ef = ef_all[:, sl].T
            z1 = hs @ whs + hwb[lo] + ef @ weate + rd @ wrd
            a1 = bfr(silu(z1 + bm1))
            z3 = a1 @ w23
            a3 = bfr(silu(z3 + b3))
            wmin = bfr(np.minimum(a3 @ wx2 + bx2, CLAMP))
            fwh = bfr(np.maximum(wmin, -CLAMP) * fd)
            conth = bfr(rel * np.tile(fwh, 3))
            np.add.at(upsum, lo, conth)
            cch += 1
        out[b * 128:(b + 1) * 128] = upsum + XPb
    return out


# ---------------------------------------------------------------- entry point

_PROGRAM_CACHE = {}


def kernel(**inputs):
    """Full-input entry: shards across 8 NeuronCores internally."""
    import sys
    for p in ("/opt/trn_rl_repo", "/root/.axon_site/_ro/trn_rl_repo"):
        if p not in sys.path:
            sys.path.append(p)
    from concourse import bass_utils

    cfg = CFG_FULL
    batch = np.asarray(inputs["batch"]).astype(np.int64)
    X = np.asarray(inputs["X"], np.float32)
    H = np.asarray(inputs["H"], np.float32)
    ei = np.asarray(inputs["edge_index"]).astype(np.int64)
    ea = np.asarray(inputs["edge_attr"], np.float32)
    te = np.asarray(inputs["te"], np.float32)

    shards, sched = build_shards(cfg, ei[0], ei[1], ea, te)
    params = make_params(cfg, *[np.asarray(inputs[k], np.float32) for k in
                         ["Wm1", "bm1", "Wm2", "bm2", "Wx1", "bx1", "Wx2",
                          "bx2", "ln_gamma", "ln_beta", "e3_weight"]])
    in_maps = [prep_core_inputs(cfg, c, shards[c], batch, X, H, params)
               for c in range(cfg.CORES)]

    key = (cfg.N, sched)
    if key not in _PROGRAM_CACHE:
        _PROGRAM_CACHE[key] = build_program(cfg, sched, cfg.CORES)
    nc = _PROGRAM_CACHE[key]

    res = bass_utils.run_bass_kernel_spmd(
        nc, in_maps, core_ids=list(range(cfg.CORES)))
    out = np.zeros((cfg.N, XW), np.float32)
    for c in range(cfg.CORES):
        out[c * cfg.NR:(c + 1) * cfg.NR] = \
            res.results[c]["OUT"][shards[c]["perm"]]
    return out.reshape(cfg.N, 3, K)
